# revision 28
# baseline (speedup 1.0000x reference)
"""Trainium2 Bass kernel for nn_LlamaAttention_61495341744411.

Sharding: tensor-parallel over heads across 8 NeuronCores.
  core c: q heads [4c, 4c+4), kv head c, wo cols [512c, 512c+512).
  Each core computes a full-token partial of out^T; host sums partials.

Device program per core (single SPMD Bass program, bf16 matmuls):
  P1: q/k/v projections from hidden^T (host-transposed), RoPE fused.
      q and k stay resident in SBUF (no DRAM spill).
  P2: causal prefill attention computed score-transposed: S^T chunks
      [keys, 4h*128q] feed exp directly into the PV matmul as lhsT —
      no P transposes, no PSUM->SBUF P copies. Softmax denominators
      come from a ones-column appended to V.
  P3: paged decode attention, KV cache pre-RoPE'd and packed on host,
      interleaved across the whole timeline (32 single-token seqs).
  P4: o_proj partial out^T = woT.T @ attn^T, emitted as PE filler
      inside the P2 windows (which are Activation-bound) + tail.

Engine discipline: every op is routed to a fixed engine so no in-order
queue ever holds an op whose dependency resolves far in the future.
"""
import sys

if "/opt/trn_rl_repo" not in sys.path:
    sys.path.insert(0, "/opt/trn_rl_repo")

import numpy as np
import ml_dtypes

BF16 = ml_dtypes.bfloat16

PREFILLS = [1024, 1536, 2048, 512]
DOFF = sum(PREFILLS)            # 5120
DECODE = 32
PAST = 2048
HIDDEN = 4096
NQ, NKV, HD = 32, 8, 128
G = NQ // NKV                   # 4
T = DOFF + DECODE               # 5152
SCALE = 1.0 / float(np.sqrt(HD))
NCORES = 8
QH = NQ // NCORES               # 4 q heads per core
ADIM = QH * HD                  # 512
KS = HIDDEN // 128              # 32 contraction subtiles
P = 128
TW = 256                        # projection t-tile width
NKT_D = PAST // P               # 16 decode cache k-tiles
NB = QH + 1                     # rope'd projection rows: [k, q0..q3]
LMAX = max(PREFILLS)

SEQ_BOUNDS = []
_off = 0
for _L in PREFILLS:
    SEQ_BOUNDS.append((_off, _L))
    _off += _L


def build_program():
    import concourse.mybir as mybir
    import concourse.tile as tile
    from concourse import bacc
    from concourse.masks import make_identity
    from contextlib import ExitStack

    dt = mybir.dt
    AF = mybir.ActivationFunctionType
    ALU = mybir.AluOpType
    f32 = dt.float32
    bf = dt.bfloat16

    nc = bacc.Bacc(None, target_bir_lowering=False, debug=False)

    hT = nc.dram_tensor("hT", [HIDDEN, T], bf, kind="ExternalInput")
    wqT = nc.dram_tensor("wqT", [HIDDEN, ADIM], bf, kind="ExternalInput")
    wkT = nc.dram_tensor("wkT", [HIDDEN, HD], bf, kind="ExternalInput")
    wvT = nc.dram_tensor("wvT", [HIDDEN, HD], bf, kind="ExternalInput")
    woT = nc.dram_tensor("woT", [ADIM, HIDDEN], bf, kind="ExternalInput")
    kTc = nc.dram_tensor("kTc", [DECODE, HD, PAST], bf, kind="ExternalInput")
    vcp = nc.dram_tensor("vcp", [DECODE, P, PAST], bf, kind="ExternalInput")
    qcos = nc.dram_tensor("qcos", [HD, T], bf, kind="ExternalInput")
    qsin = nc.dram_tensor("qsin", [HD, T], bf, kind="ExternalInput")
    outT = nc.dram_tensor("outT", [HIDDEN, T], bf, kind="ExternalOutput")
    import os as _os
    DBG = bool(_os.environ.get("KBDBG"))
    if DBG:
        dbg_q = nc.dram_tensor("dbg_q", [P, QH, 1024], bf, kind="ExternalOutput")
        dbg_k = nc.dram_tensor("dbg_k", [P, 1024], bf, kind="ExternalOutput")
        dbg_v = nc.dram_tensor("dbg_v", [P, 8, HD + 1], bf, kind="ExternalOutput")
        dbg_a = nc.dram_tensor("dbg_a", [P, QH, 1024], bf, kind="ExternalOutput")

    hT_r = hT.rearrange("(o p) t -> p o t", p=P)        # [128, 32, T]
    wqT_r = wqT.rearrange("(o p) m -> p o m", p=P)      # [128, 32, 512]
    wkT_r = wkT.rearrange("(o p) m -> p o m", p=P)      # [128, 32, 128]
    wvT_r = wvT.rearrange("(o p) m -> p o m", p=P)
    woT_r = woT.rearrange("(o p) m -> p o m", p=P)      # [128, 4, 4096]
    outT_r = outT.rearrange("(o p) t -> p o t", p=P)    # [128, 32, T]

    with ExitStack() as ctx:
        tc = ctx.enter_context(tile.TileContext(nc))
        p1 = ctx.enter_context(tc.tile_pool(name="p1", bufs=1))
        p2 = ctx.enter_context(tc.tile_pool(name="p2", bufs=2))
        p3 = ctx.enter_context(tc.tile_pool(name="p3", bufs=3))
        p4 = ctx.enter_context(tc.tile_pool(name="p4", bufs=4))
        pDecK = ctx.enter_context(tc.tile_pool(name="pDecK", bufs=3))
        pDecV = ctx.enter_context(tc.tile_pool(name="pDecV", bufs=2))
        psBig = ctx.enter_context(tc.tile_pool(name="psBig", bufs=3,
                                               space="PSUM"))
        psPo = ctx.enter_context(tc.tile_pool(name="psPo", bufs=1,
                                              space="PSUM"))
        psSm = ctx.enter_context(tc.tile_pool(name="psSm", bufs=1,
                                              space="PSUM"))

        ident = p1.tile([P, P], bf, tag="ident")
        make_identity(nc, ident)
        ones_sb = p1.tile([P, 1], bf, tag="ones")
        nc.vector.memset(ones_sb[:], 1.0)
        ones_row = p1.tile([1, P], bf, tag="ones_row")
        nc.vector.memset(ones_row[:], 1.0)

        # resident weights (wq/wo DMAs deferred below so the first
        # ht tiles aren't starved on the shared DMA path)
        wk_sb = p1.tile([P, KS, HD], bf, tag="wk")
        nc.sync.dma_start(wk_sb[:], wkT_r[:])
        wv_sb = p1.tile([P, KS, HD], bf, tag="wv")
        nc.sync.dma_start(wv_sb[:], wvT_r[:])
        wq_sb = p1.tile([P, KS, ADIM], bf, tag="wq")
        wo_sb = p1.tile([P, QH, HIDDEN], bf, tag="wo")

        # resident per-seq activations (single slot, reused across seqs)
        q_sb = p1.tile([P, QH, LMAX], bf, tag="q_sb")
        attn_A = p1.tile([P, QH, 1024], bf, tag="attn_A")
        attn_B = p1.tile([P, QH, LMAX], bf, tag="attn_B")
        kT_sb = p1.tile([P, LMAX], bf, tag="kT_sb")
        v_nat = p1.tile([P, LMAX // P, HD + 1], bf, tag="v_nat")

        # decode residents
        kT_dec = p1.tile([P, DECODE], bf, tag="kTdec")
        vdt = p1.tile([DECODE, HD], bf, tag="vdt")
        qdec_sb = p1.tile([P, P], bf, tag="qdec")      # cols (s,h)
        odec_sb = p1.tile([P, HD], bf, tag="odec")     # rows (s,h)
        attn_dec = p1.tile([P, QH, DECODE], bf, tag="attn_dec")

        qdec_r = qdec_sb.rearrange("p (s h) -> p s h", h=QH)

        # ---------------- phase 1: projections + rope ----------------
        def phase1_tile(t0, W, kT_dst, kcol0, v_dst, q_dst, qcol0,
                        dec=False, wq_dma=None, split_ht=1):
            H2 = KS // 2
            hta = p2.tile([P, H2, TW], bf, tag="hta")
            htb = p2.tile([P, H2, TW], bf, tag="htb")
            for half, ht_t in ((0, hta), (1, htb)):
                for i in range(split_ht):
                    sl = slice(i * H2 // split_ht, (i + 1) * H2 // split_ht)
                    sl2 = slice(half * H2 + i * H2 // split_ht,
                                half * H2 + (i + 1) * H2 // split_ht)
                    nc.sync.dma_start(ht_t[:, sl, :W], hT_r[:, sl2, t0:t0 + W])
            ct = p2.tile([P, TW], bf, tag="cos")
            st = p2.tile([P, TW], bf, tag="sin")
            nc.gpsimd.dma_start(ct[:, :W], qcos[:, t0:t0 + W])
            nc.gpsimd.dma_start(st[:, :W], qsin[:, t0:t0 + W])

            xq = p2.tile([P, NB, TW], bf, tag="xq")

            def group(w_ap, sink):
                ps = psBig.tile([P, 512], f32, tag="big")
                for ks in range(KS):
                    src = hta if ks < KS // 2 else htb
                    nc.tensor.matmul(
                        ps[:, :W], lhsT=w_ap[:, ks, :],
                        rhs=src[:, ks % (KS // 2), :W],
                        start=(ks == 0), stop=(ks == KS - 1),
                        skip_group_check=True)
                sink(ps)

            # k group -> xq row 0
            group(wk_sb, lambda ps: nc.scalar.activation(
                xq[:, 0, :W], ps[:, :W], AF.Copy))

            # v group -> transposes into v_dst
            def v_sink(ps):
                vt = p3.tile([P, TW], bf, tag="vt")
                nc.vector.tensor_copy(out=vt[:, :W], in_=ps[:, :W])
                if not dec:
                    for j in range(W // P):
                        pst = psSm.tile([P, P], bf, tag="sm")
                        nc.tensor.transpose(
                            pst[:], vt[:, j * P:(j + 1) * P], ident[:])
                        nc.vector.tensor_copy(
                            out=v_dst[:, kcol0 // P + j, :HD], in_=pst[:])
                else:
                    pst = psSm.tile([P, P], bf, tag="sm")
                    nc.tensor.transpose(pst[:DECODE, :], vt[:, :W], ident[:])
                    nc.vector.tensor_copy(out=v_dst[:], in_=pst[:DECODE, :])
            group(wv_sb, v_sink)

            if wq_dma is not None:
                wsrc, nchunks = wq_dma
                half = ADIM // nchunks
                for i in range(nchunks):
                    nc.sync.dma_start(
                        wq_sb[:, :, i * half:(i + 1) * half],
                        wsrc[:, :, i * half:(i + 1) * half])

            # q groups -> xq rows 1..4
            for m in range(QH):
                group(wq_sb[:, :, m * P:(m + 1) * P],
                      lambda ps, m=m: nc.scalar.activation(
                          xq[:, 1 + m, :W], ps[:, :W], AF.Copy))

            # rope on xq rows (k + 4 q)
            rotq = p2.tile([P, NB, TW], bf, tag="rotq")
            nc.gpsimd.dma_start(out=rotq[0:64, :, :W], in_=xq[64:128, :, :W])
            nc.gpsimd.dma_start(out=rotq[64:128, :, :W], in_=xq[0:64, :, :W])
            ct_b = ct[:, None, :W].to_broadcast((P, NB, W))
            st_b = st[:, None, :W].to_broadcast((P, NB, W))
            nc.vector.tensor_tensor(xq[:, :, :W], xq[:, :, :W], ct_b,
                                    ALU.mult)
            nc.vector.tensor_tensor(rotq[:, :, :W], rotq[:, :, :W], st_b,
                                    ALU.mult)
            nc.vector.tensor_tensor(
                kT_dst[:, kcol0:kcol0 + W], xq[:, 0, :W], rotq[:, 0, :W],
                ALU.add)
            nc.vector.tensor_tensor(
                q_dst[:, :, qcol0:qcol0 + W], xq[:, 1:NB, :W],
                rotq[:, 1:NB, :W], ALU.add)

        # ---------------- phase 4 unit generator ----------------
        NOMB = 2
        omb = {"buf": None, "fill": 0}
        ph4_in_window = [True]

        def phase4_units(out0, src0, L, src):
            """Yield once per m-block (4 matmuls + copy [+dma])."""
            for t0 in range(0, L, 512):
                W = min(512, L - t0)
                for mg in range(KS):
                    ps = psBig.tile([P, 512], f32, tag="big")
                    for ks in range(QH):
                        nc.tensor.matmul(
                            ps[:, :W], lhsT=wo_sb[:, ks, mg * P:(mg + 1) * P],
                            rhs=src[:, ks, src0 + t0:src0 + t0 + W],
                            start=(ks == 0), stop=(ks == QH - 1),
                            skip_group_check=True)
                    if omb["buf"] is None:
                        omb["n"] = omb.get("n", 0) + 1
                        omb["buf"] = p2.tile([P, NOMB, 512], bf, tag="omb",
                                             name=f"omb{omb['n']}")
                        omb["fill"] = 0
                    ob = omb["buf"]
                    slot = omb["fill"]
                    if ph4_in_window[0] or mg % 2 == 0:
                        nc.vector.tensor_copy(out=ob[:, slot, :W],
                                              in_=ps[:, :W])
                    else:
                        nc.scalar.activation(ob[:, slot, :W], ps[:, :W],
                                             AF.Copy)
                    omb["fill"] += 1
                    if omb["fill"] == NOMB:
                        nc.sync.dma_start(
                            outT_r[:, mg - NOMB + 1:mg + 1,
                                   out0 + t0:out0 + t0 + W],
                            ob[:, :, :W])
                        omb["buf"] = None
                    yield

        ph4_queue = []

        def pull_ph4(n):
            for _ in range(n):
                if not ph4_queue:
                    return
                try:
                    next(ph4_queue[0][1])
                except StopIteration:
                    ph4_queue.pop(0)

        def drain_older_than(idx):
            while ph4_queue and ph4_queue[0][0] < idx:
                try:
                    next(ph4_queue[0][1])
                except StopIteration:
                    ph4_queue.pop(0)

        # ---------------- decode units ----------------
        def prefetch_kd(s):
            kd = pDecK.tile([P, PAST], bf, tag="kd", name=f"kd{s}")
            nc.sync.dma_start(kd[:], kTc[s])
            return kd

        def prefetch_vd(s):
            vd = pDecV.tile([P, NKT_D, HD], bf, tag="vd", name=f"vd{s}")
            nc.sync.dma_start(vd[:], vcp[s].rearrange("p (kt d) -> p kt d",
                                                      d=HD))
            return vd

        dec_state = {"next": 0, "tiles": {}}

        def decode_emit_one():
            s = dec_state["next"]
            if s >= DECODE:
                return False
            dec_state["next"] += 1
            ks_, vs_ = dec_state.setdefault("kd", {}), dec_state.setdefault(
                "vd", {})
            if s == 0:
                ks_[0] = prefetch_kd(0)
                ks_[1] = prefetch_kd(1)
                vs_[0] = prefetch_vd(0)
            if s + 2 < DECODE:
                ks_[s + 2] = prefetch_kd(s + 2)
            if s + 1 < DECODE:
                vs_[s + 1] = prefetch_vd(s + 1)
            kd, vd = ks_.pop(s), vs_.pop(s)

            stp = psBig.tile([P, 72], f32, tag="big", name=f"stp{s}")
            for kt in range(NKT_D):
                nc.tensor.matmul(
                    stp[:, kt * QH:(kt + 1) * QH],
                    lhsT=kd[:, kt * P:(kt + 1) * P],
                    rhs=qdec_sb[:, s * QH:(s + 1) * QH],
                    start=True, stop=True, skip_group_check=True)
            nc.tensor.matmul(
                stp[0:1, 64:68], lhsT=kT_dec[:, s:s + 1],
                rhs=qdec_sb[:, s * QH:(s + 1) * QH],
                start=True, stop=True, skip_group_check=True)
            pt = p3.tile([P, 72], bf, tag="ptd")
            nc.scalar.activation(pt[:, :64], stp[:, :64], AF.Exp, scale=SCALE)
            nc.scalar.activation(pt[0:1, 64:68], stp[0:1, 64:68], AF.Exp,
                                 scale=SCALE)

            vrow = p3.tile([1, HD], bf, tag="vrow")
            nc.gpsimd.dma_start(out=vrow[:], in_=vdt[s:s + 1, :])
            # PV transposed: out [128 d, 4 h] so each matmul is 4 rows
            ov = psBig.tile([P, QH], f32, tag="big", name=f"ov{s}")
            ovs = psSm.tile([1, QH], f32, tag="sm", name=f"ovs{s}")
            for kt in range(NKT_D):
                nc.tensor.matmul(
                    ov[:], lhsT=vd[:, kt, :],
                    rhs=pt[:, kt * QH:(kt + 1) * QH],
                    start=(kt == 0), stop=False, skip_group_check=True)
                nc.tensor.matmul(
                    ovs[:], lhsT=ones_sb[:],
                    rhs=pt[:, kt * QH:(kt + 1) * QH],
                    start=(kt == 0), stop=False, skip_group_check=True)
            nc.tensor.matmul(ov[:], lhsT=vrow[:],
                             rhs=pt[0:1, 64:68], start=False, stop=True,
                             skip_group_check=True)
            nc.tensor.matmul(ovs[:], lhsT=ones_sb[0:1, :],
                             rhs=pt[0:1, 64:68], start=False, stop=True,
                             skip_group_check=True)
            # reciprocal row, then PE-broadcast it down all 128 partitions
            rrow = p3.tile([1, QH], bf, tag="rrow")
            with nc.allow_low_precision(reason="bf16 softmax denom"):
                nc.vector.reciprocal(rrow[:], ovs[:])
            rbc = psSm.tile([P, QH], f32, tag="sm", name=f"rbc{s}")
            nc.tensor.matmul(rbc[:], lhsT=ones_row[:], rhs=rrow[:],
                             start=True, stop=True, skip_group_check=True)
            rbs = p3.tile([P, QH], bf, tag="rbs")
            nc.vector.tensor_copy(out=rbs[:], in_=rbc[:])
            nc.vector.tensor_tensor(attn_dec[:, :, s], ov[:], rbs[:],
                                    ALU.mult)
            if s == DECODE - 1:
                ph4_queue.append(
                    (98, phase4_units(DOFF, 0, DECODE, attn_dec)))
            return True

        def enqueue_ph4(idx, si, t0, L):
            s0, _ = SEQ_BOUNDS[si]
            slot = attn_A if idx % 2 == 0 else attn_B
            ph4_queue.append((idx, phase4_units(s0 + t0, t0, L, slot)))

        # ---------------- phase 2 window ----------------
        def window(idx, si):
            s0, L = SEQ_BOUNDS[si]
            attn_sb = attn_A if idx % 2 == 0 else attn_B
            B = L // P
            debt = [0.0]

            def fill(ns):
                debt[0] += ns
                while debt[0] >= 853.0 and ph4_queue:
                    try:
                        next(ph4_queue[0][1])
                        debt[0] -= 853.0
                    except StopIteration:
                        ph4_queue.pop(0)

            prev = {"pos": None, "Q0": 0, "obufs": None}

            def emit_norm():
                # previous block's softmax normalization (DVE only)
                obufs = []
                for h in range(QH):
                    rr = p3.tile([P, 1], f32, tag="rr")
                    nc.vector.reciprocal(rr[:], prev["pos"][h][:, HD:HD + 1])
                    obuf = p4.tile([P, P], bf, tag="obuf",
                                   name=f"ob{si}_{prev['Q0']}_{h}")
                    nc.vector.tensor_scalar_mul(
                        obuf[:], prev["pos"][h][:, :HD], rr[:])
                    obufs.append(obuf)
                prev["obufs"] = obufs
                prev["pos"] = None

            def emit_transposes():
                Q0p = prev["Q0"]
                for h in range(QH):
                    pst = psSm.tile([P, P], bf, tag="sm")
                    nc.tensor.transpose(pst[:], prev["obufs"][h][:], ident[:])
                    if h % 2 == 0:
                        nc.vector.tensor_copy(
                            out=attn_sb[:, h, Q0p:Q0p + P], in_=pst[:])
                    else:
                        nc.scalar.activation(
                            attn_sb[:, h, Q0p:Q0p + P], pst[:], AF.Copy)
                prev["obufs"] = None
                if (Q0p // P + 1) % 4 == 0:
                    enqueue_ph4(idx, si, Q0p - 3 * P, 512)

            for qb in range(B):
                Q0 = qb * P
                pos = [psPo.tile([P, HD + 1], f32, tag=f"po{h}",
                                 name=f"po{h}_{si}_{qb}") for h in range(QH)]
                q4 = q_sb[:, :, Q0:Q0 + P]
                pending = []   # (pch, c) with exp done, PV not yet emitted

                def emit_pv(pch, c, qb=qb, pos=pos):
                    for h in range(QH):
                        nc.tensor.matmul(
                            pos[h][:], lhsT=pch[:, h, :],
                            rhs=v_nat[:, c, :], start=(c == 0),
                            stop=(c == qb), skip_group_check=True)

                for c in range(qb + 1):
                    sps = psBig.tile([P, 512], f32, tag="big")
                    nc.tensor.matmul(sps[:], lhsT=kT_sb[:, c * P:(c + 1) * P],
                                     rhs=q4, start=True, stop=True,
                                     skip_group_check=True)
                    pch = p4.tile([P, QH, P], bf, tag="pch")
                    nc.scalar.activation(pch[:], sps[:], AF.Exp, scale=SCALE)
                    if c == qb:
                        nc.gpsimd.affine_select(
                            out=pch[:], in_=pch[:], compare_op=ALU.is_ge,
                            fill=0.0, base=0, channel_multiplier=-1,
                            pattern=[[0, QH], [1, P]])
                    if c == 0 and prev["pos"] is not None:
                        emit_norm()
                    if len(pending) >= 2:
                        emit_pv(*pending.pop(0))
                    pending.append((pch, c))
                    if c == 1 and prev["obufs"] is not None:
                        emit_transposes()
                    fill(450.0)
                while pending:
                    emit_pv(*pending.pop(0))
                if prev["obufs"] is not None:   # block had < 2 chunks
                    emit_transposes()
                prev["pos"] = pos
                prev["Q0"] = Q0
                decode_emit_one()
            # final block: norm + transposes at window end
            emit_norm()
            emit_transposes()

        # ---------------- program ----------------
        # ones column of v_nat (rewritten per seq by transposes on :HD only)
        nc.vector.memset(v_nat[:, :, HD:HD + 1], 1.0)

        # seq0 tile 0 first (k/v groups run on wk/wv alone), weight DMAs
        # threaded between so the shared DMA path never starves PE
        qd_tmp = p1.tile([P, QH, DECODE], bf, tag="qd_tmp")
        phase1_tile(0, TW, kT_sb, 0, v_nat, q_sb, 0,
                    wq_dma=(wqT_r, 2), split_ht=4)
        phase1_tile(DOFF, DECODE, kT_dec, 0, vdt, qd_tmp, 0, dec=True)
        for h in range(QH):
            nc.gpsimd.dma_start(out=qdec_r[:, :, h], in_=qd_tmp[:, h, :])

        first_tile_done = True
        for idx, si in enumerate([0, 1, 3, 2]):
            s0, L = SEQ_BOUNDS[si]
            for lt in range(L // TW):
                if si == 0 and lt == 0:
                    continue
                phase1_tile(s0 + lt * TW, TW, kT_sb, lt * TW, v_nat,
                            q_sb, lt * TW)
                decode_emit_one()
                if lt == 1 and idx == 0:
                    nc.sync.dma_start(wo_sb[:], woT_r[:])
            # phase4 of the seq that used this window's attn slot last
            # must be fully drained before this window's first attn write
            drain_older_than(idx - 1)
            window(idx, si)

        ph4_in_window[0] = False
        while ph4_queue:
            pull_ph4(64)
        ph4_in_window[0] = True

        # remaining decode units (assembly handled inside the last unit)
        while decode_emit_one():
            pass
        while ph4_queue:
            pull_ph4(64)

    nc.compile()
    return nc


_NC = None


def _get_program():
    global _NC
    if _NC is None:
        _NC = build_program()
    return _NC


def _rope_tables():
    inv_freq = 1.0 / (10000.0 ** (np.arange(0, HD, 2, dtype=np.float32) / HD))
    pos_q = np.concatenate(
        [np.arange(L, dtype=np.float32) for L in PREFILLS]
        + [np.full(DECODE, float(PAST), np.float32)])                 # [T]
    ang_q = np.outer(inv_freq, pos_q)                                 # [64, T]
    qcos = np.concatenate([np.cos(ang_q), np.cos(ang_q)], axis=0)
    qsin = np.concatenate([-np.sin(ang_q), np.sin(ang_q)], axis=0)
    return qcos.astype(BF16), qsin.astype(BF16)


def _rope_cache(k):
    # k: [DECODE, PAST, HD] float32; positions 0..PAST-1
    inv_freq = 1.0 / (10000.0 ** (np.arange(0, HD, 2, dtype=np.float32) / HD))
    ang = np.outer(np.arange(PAST, dtype=np.float32), inv_freq)  # [PAST, 64]
    cos = np.concatenate([np.cos(ang), np.cos(ang)], axis=1)     # [PAST, 128]
    sin = np.concatenate([np.sin(ang), np.sin(ang)], axis=1)
    rot = np.concatenate([-k[..., HD // 2:], k[..., :HD // 2]], axis=-1)
    return k * cos[None] + rot * sin[None]


def make_in_maps(hidden_states, wq, wk, wv, wo, kv_cache_k, kv_cache_v):
    hidden_states = np.asarray(hidden_states)
    wq, wk, wv, wo = (np.asarray(a) for a in (wq, wk, wv, wo))
    kv_cache_k, kv_cache_v = np.asarray(kv_cache_k), np.asarray(kv_cache_v)

    hT = np.ascontiguousarray(hidden_states.astype(BF16).T)      # [4096, T]
    qcos, qsin = _rope_tables()
    in_maps = []
    for c in range(NCORES):
        wqT = np.ascontiguousarray(wq[c * ADIM:(c + 1) * ADIM, :]
                                   .astype(BF16).T)
        wkT = np.ascontiguousarray(wk[c * HD:(c + 1) * HD, :].astype(BF16).T)
        wvT = np.ascontiguousarray(wv[c * HD:(c + 1) * HD, :].astype(BF16).T)
        woT = np.ascontiguousarray(wo[:, c * ADIM:(c + 1) * ADIM]
                                   .astype(BF16).T)
        kc = _rope_cache(kv_cache_k[:, :, c, :].astype(np.float32))
        kTc = np.ascontiguousarray(
            kc.astype(BF16).transpose(0, 2, 1))                  # [32,128,2048]
        vcc = kv_cache_v[:, :, c, :].astype(BF16)                # [32,2048,128]
        vcp = np.ascontiguousarray(
            vcc.reshape(DECODE, NKT_D, P, HD).transpose(0, 2, 1, 3)
            .reshape(DECODE, P, PAST))
        in_maps.append({
            "hT": hT, "wqT": wqT, "wkT": wkT, "wvT": wvT, "woT": woT,
            "kTc": kTc, "vcp": vcp, "qcos": qcos, "qsin": qsin,
        })
    return in_maps


def combine_outputs(results):
    acc = np.zeros((HIDDEN, T), np.float32)
    for c in range(NCORES):
        acc += results[c]["outT"].astype(np.float32)
    return np.ascontiguousarray(acc.T)


def kernel(hidden_states, wq, wk, wv, wo, kv_cache_k, kv_cache_v):
    from concourse.bass_utils import run_bass_kernel_spmd

    nc = _get_program()
    in_maps = make_in_maps(hidden_states, wq, wk, wv, wo,
                           kv_cache_k, kv_cache_v)
    res = run_bass_kernel_spmd(nc, in_maps, core_ids=list(range(NCORES)))
    return combine_outputs(res.results)


# revision 29
# speedup vs baseline: 1.0002x; 1.0002x over previous
"""Trainium2 Bass kernel for nn_LlamaAttention_61495341744411.

Sharding: tensor-parallel over heads across 8 NeuronCores.
  core c: q heads [4c, 4c+4), kv head c, wo cols [512c, 512c+512).
  Each core computes a full-token partial of out^T; host sums partials.

Device program per core (single SPMD Bass program, bf16 matmuls):
  P1: q/k/v projections from hidden^T (host-transposed), RoPE fused.
      q and k stay resident in SBUF (no DRAM spill).
  P2: causal prefill attention computed score-transposed: S^T chunks
      [keys, 4h*128q] feed exp directly into the PV matmul as lhsT —
      no P transposes, no PSUM->SBUF P copies. Softmax denominators
      come from a ones-column appended to V.
  P3: paged decode attention, KV cache pre-RoPE'd and packed on host,
      interleaved across the whole timeline (32 single-token seqs).
  P4: o_proj partial out^T = woT.T @ attn^T, emitted as PE filler
      inside the P2 windows (which are Activation-bound) + tail.

Engine discipline: every op is routed to a fixed engine so no in-order
queue ever holds an op whose dependency resolves far in the future.
"""
import sys

if "/opt/trn_rl_repo" not in sys.path:
    sys.path.insert(0, "/opt/trn_rl_repo")

import numpy as np
import ml_dtypes

BF16 = ml_dtypes.bfloat16

PREFILLS = [1024, 1536, 2048, 512]
DOFF = sum(PREFILLS)            # 5120
DECODE = 32
PAST = 2048
HIDDEN = 4096
NQ, NKV, HD = 32, 8, 128
G = NQ // NKV                   # 4
T = DOFF + DECODE               # 5152
SCALE = 1.0 / float(np.sqrt(HD))
NCORES = 8
QH = NQ // NCORES               # 4 q heads per core
ADIM = QH * HD                  # 512
KS = HIDDEN // 128              # 32 contraction subtiles
P = 128
TW = 256                        # projection t-tile width
NKT_D = PAST // P               # 16 decode cache k-tiles
NB = QH + 1                     # rope'd projection rows: [k, q0..q3]
LMAX = max(PREFILLS)

SEQ_BOUNDS = []
_off = 0
for _L in PREFILLS:
    SEQ_BOUNDS.append((_off, _L))
    _off += _L


def build_program():
    import concourse.mybir as mybir
    import concourse.tile as tile
    from concourse import bacc
    from concourse.masks import make_identity
    from contextlib import ExitStack

    dt = mybir.dt
    AF = mybir.ActivationFunctionType
    ALU = mybir.AluOpType
    f32 = dt.float32
    bf = dt.bfloat16

    nc = bacc.Bacc(None, target_bir_lowering=False, debug=False)

    hT = nc.dram_tensor("hT", [HIDDEN, T], bf, kind="ExternalInput")
    wqT = nc.dram_tensor("wqT", [HIDDEN, ADIM], bf, kind="ExternalInput")
    wkT = nc.dram_tensor("wkT", [HIDDEN, HD], bf, kind="ExternalInput")
    wvT = nc.dram_tensor("wvT", [HIDDEN, HD], bf, kind="ExternalInput")
    woT = nc.dram_tensor("woT", [ADIM, HIDDEN], bf, kind="ExternalInput")
    kTc = nc.dram_tensor("kTc", [DECODE, HD, PAST], bf, kind="ExternalInput")
    vcp = nc.dram_tensor("vcp", [DECODE, P, PAST], bf, kind="ExternalInput")
    qcos = nc.dram_tensor("qcos", [HD, T], bf, kind="ExternalInput")
    qsin = nc.dram_tensor("qsin", [HD, T], bf, kind="ExternalInput")
    outT = nc.dram_tensor("outT", [HIDDEN, T], bf, kind="ExternalOutput")
    import os as _os
    DBG = bool(_os.environ.get("KBDBG"))
    if DBG:
        dbg_q = nc.dram_tensor("dbg_q", [P, QH, 1024], bf, kind="ExternalOutput")
        dbg_k = nc.dram_tensor("dbg_k", [P, 1024], bf, kind="ExternalOutput")
        dbg_v = nc.dram_tensor("dbg_v", [P, 8, HD + 1], bf, kind="ExternalOutput")
        dbg_a = nc.dram_tensor("dbg_a", [P, QH, 1024], bf, kind="ExternalOutput")

    hT_r = hT.rearrange("(o p) t -> p o t", p=P)        # [128, 32, T]
    wqT_r = wqT.rearrange("(o p) m -> p o m", p=P)      # [128, 32, 512]
    wkT_r = wkT.rearrange("(o p) m -> p o m", p=P)      # [128, 32, 128]
    wvT_r = wvT.rearrange("(o p) m -> p o m", p=P)
    woT_r = woT.rearrange("(o p) m -> p o m", p=P)      # [128, 4, 4096]
    outT_r = outT.rearrange("(o p) t -> p o t", p=P)    # [128, 32, T]

    with ExitStack() as ctx:
        tc = ctx.enter_context(tile.TileContext(nc))
        p1 = ctx.enter_context(tc.tile_pool(name="p1", bufs=1))
        p2 = ctx.enter_context(tc.tile_pool(name="p2", bufs=2))
        p3 = ctx.enter_context(tc.tile_pool(name="p3", bufs=3))
        p4 = ctx.enter_context(tc.tile_pool(name="p4", bufs=4))
        pDecK = ctx.enter_context(tc.tile_pool(name="pDecK", bufs=3))
        pDecV = ctx.enter_context(tc.tile_pool(name="pDecV", bufs=2))
        psBig = ctx.enter_context(tc.tile_pool(name="psBig", bufs=3,
                                               space="PSUM"))
        psPo = ctx.enter_context(tc.tile_pool(name="psPo", bufs=1,
                                              space="PSUM"))
        psSm = ctx.enter_context(tc.tile_pool(name="psSm", bufs=1,
                                              space="PSUM"))

        ident = p1.tile([P, P], bf, tag="ident")
        make_identity(nc, ident)
        ones_sb = p1.tile([P, 1], bf, tag="ones")
        nc.vector.memset(ones_sb[:], 1.0)
        ones_row = p1.tile([1, P], bf, tag="ones_row")
        nc.vector.memset(ones_row[:], 1.0)

        # resident weights (wq/wo DMAs deferred below so the first
        # ht tiles aren't starved on the shared DMA path)
        wk_sb = p1.tile([P, KS, HD], bf, tag="wk")
        nc.sync.dma_start(wk_sb[:], wkT_r[:])
        wv_sb = p1.tile([P, KS, HD], bf, tag="wv")
        wq_sb = p1.tile([P, KS, ADIM], bf, tag="wq")
        wo_sb = p1.tile([P, QH, HIDDEN], bf, tag="wo")

        # resident per-seq activations (single slot, reused across seqs)
        q_sb = p1.tile([P, QH, LMAX], bf, tag="q_sb")
        attn_A = p1.tile([P, QH, 1024], bf, tag="attn_A")
        attn_B = p1.tile([P, QH, LMAX], bf, tag="attn_B")
        kT_sb = p1.tile([P, LMAX], bf, tag="kT_sb")
        v_nat = p1.tile([P, LMAX // P, HD + 1], bf, tag="v_nat")

        # decode residents
        kT_dec = p1.tile([P, DECODE], bf, tag="kTdec")
        vdt = p1.tile([DECODE, HD], bf, tag="vdt")
        qdec_sb = p1.tile([P, P], bf, tag="qdec")      # cols (s,h)
        odec_sb = p1.tile([P, HD], bf, tag="odec")     # rows (s,h)
        attn_dec = p1.tile([P, QH, DECODE], bf, tag="attn_dec")

        qdec_r = qdec_sb.rearrange("p (s h) -> p s h", h=QH)

        # ---------------- phase 1: projections + rope ----------------
        def phase1_tile(t0, W, kT_dst, kcol0, v_dst, q_dst, qcol0,
                        dec=False, wq_dma=None, split_ht=1):
            H2 = KS // 2
            hta = p2.tile([P, H2, TW], bf, tag="hta")
            htb = p2.tile([P, H2, TW], bf, tag="htb")
            for half, ht_t in ((0, hta), (1, htb)):
                for i in range(split_ht):
                    sl = slice(i * H2 // split_ht, (i + 1) * H2 // split_ht)
                    sl2 = slice(half * H2 + i * H2 // split_ht,
                                half * H2 + (i + 1) * H2 // split_ht)
                    nc.sync.dma_start(ht_t[:, sl, :W], hT_r[:, sl2, t0:t0 + W])
            if split_ht > 1:
                nc.sync.dma_start(wv_sb[:], wvT_r[:])
            ct = p2.tile([P, TW], bf, tag="cos")
            st = p2.tile([P, TW], bf, tag="sin")
            nc.gpsimd.dma_start(ct[:, :W], qcos[:, t0:t0 + W])
            nc.gpsimd.dma_start(st[:, :W], qsin[:, t0:t0 + W])

            xq = p2.tile([P, NB, TW], bf, tag="xq")

            def group(w_ap, sink):
                ps = psBig.tile([P, 512], f32, tag="big")
                for ks in range(KS):
                    src = hta if ks < KS // 2 else htb
                    nc.tensor.matmul(
                        ps[:, :W], lhsT=w_ap[:, ks, :],
                        rhs=src[:, ks % (KS // 2), :W],
                        start=(ks == 0), stop=(ks == KS - 1),
                        skip_group_check=True)
                sink(ps)

            # k group -> xq row 0
            group(wk_sb, lambda ps: nc.scalar.activation(
                xq[:, 0, :W], ps[:, :W], AF.Copy))

            # v group -> transposes into v_dst
            def v_sink(ps):
                vt = p3.tile([P, TW], bf, tag="vt")
                nc.vector.tensor_copy(out=vt[:, :W], in_=ps[:, :W])
                if not dec:
                    for j in range(W // P):
                        pst = psSm.tile([P, P], bf, tag="sm")
                        nc.tensor.transpose(
                            pst[:], vt[:, j * P:(j + 1) * P], ident[:])
                        nc.vector.tensor_copy(
                            out=v_dst[:, kcol0 // P + j, :HD], in_=pst[:])
                else:
                    pst = psSm.tile([P, P], bf, tag="sm")
                    nc.tensor.transpose(pst[:DECODE, :], vt[:, :W], ident[:])
                    nc.vector.tensor_copy(out=v_dst[:], in_=pst[:DECODE, :])
            group(wv_sb, v_sink)

            if wq_dma is not None:
                wsrc, nchunks = wq_dma
                half = ADIM // nchunks
                for i in range(nchunks):
                    nc.sync.dma_start(
                        wq_sb[:, :, i * half:(i + 1) * half],
                        wsrc[:, :, i * half:(i + 1) * half])

            # q groups -> xq rows 1..4
            for m in range(QH):
                group(wq_sb[:, :, m * P:(m + 1) * P],
                      lambda ps, m=m: nc.scalar.activation(
                          xq[:, 1 + m, :W], ps[:, :W], AF.Copy))

            # rope on xq rows (k + 4 q)
            rotq = p2.tile([P, NB, TW], bf, tag="rotq")
            nc.gpsimd.dma_start(out=rotq[0:64, :, :W], in_=xq[64:128, :, :W])
            nc.gpsimd.dma_start(out=rotq[64:128, :, :W], in_=xq[0:64, :, :W])
            ct_b = ct[:, None, :W].to_broadcast((P, NB, W))
            st_b = st[:, None, :W].to_broadcast((P, NB, W))
            nc.vector.tensor_tensor(xq[:, :, :W], xq[:, :, :W], ct_b,
                                    ALU.mult)
            nc.vector.tensor_tensor(rotq[:, :, :W], rotq[:, :, :W], st_b,
                                    ALU.mult)
            nc.vector.tensor_tensor(
                kT_dst[:, kcol0:kcol0 + W], xq[:, 0, :W], rotq[:, 0, :W],
                ALU.add)
            nc.vector.tensor_tensor(
                q_dst[:, :, qcol0:qcol0 + W], xq[:, 1:NB, :W],
                rotq[:, 1:NB, :W], ALU.add)

        # ---------------- phase 4 unit generator ----------------
        NOMB = 2
        omb = {"buf": None, "fill": 0}
        ph4_in_window = [True]

        def phase4_units(out0, src0, L, src):
            """Yield once per m-block (4 matmuls + copy [+dma])."""
            for t0 in range(0, L, 512):
                W = min(512, L - t0)
                for mg in range(KS):
                    ps = psBig.tile([P, 512], f32, tag="big")
                    for ks in range(QH):
                        nc.tensor.matmul(
                            ps[:, :W], lhsT=wo_sb[:, ks, mg * P:(mg + 1) * P],
                            rhs=src[:, ks, src0 + t0:src0 + t0 + W],
                            start=(ks == 0), stop=(ks == QH - 1),
                            skip_group_check=True)
                    if omb["buf"] is None:
                        omb["n"] = omb.get("n", 0) + 1
                        omb["buf"] = p2.tile([P, NOMB, 512], bf, tag="omb",
                                             name=f"omb{omb['n']}")
                        omb["fill"] = 0
                    ob = omb["buf"]
                    slot = omb["fill"]
                    if ph4_in_window[0] or mg % 2 == 0:
                        nc.vector.tensor_copy(out=ob[:, slot, :W],
                                              in_=ps[:, :W])
                    else:
                        nc.scalar.activation(ob[:, slot, :W], ps[:, :W],
                                             AF.Copy)
                    omb["fill"] += 1
                    if omb["fill"] == NOMB:
                        nc.sync.dma_start(
                            outT_r[:, mg - NOMB + 1:mg + 1,
                                   out0 + t0:out0 + t0 + W],
                            ob[:, :, :W])
                        omb["buf"] = None
                    yield

        ph4_queue = []

        def pull_ph4(n):
            for _ in range(n):
                if not ph4_queue:
                    return
                try:
                    next(ph4_queue[0][1])
                except StopIteration:
                    ph4_queue.pop(0)

        def drain_older_than(idx):
            while ph4_queue and ph4_queue[0][0] < idx:
                try:
                    next(ph4_queue[0][1])
                except StopIteration:
                    ph4_queue.pop(0)

        # ---------------- decode units ----------------
        def prefetch_kd(s):
            kd = pDecK.tile([P, PAST], bf, tag="kd", name=f"kd{s}")
            nc.sync.dma_start(kd[:], kTc[s])
            return kd

        def prefetch_vd(s):
            vd = pDecV.tile([P, NKT_D, HD], bf, tag="vd", name=f"vd{s}")
            nc.sync.dma_start(vd[:], vcp[s].rearrange("p (kt d) -> p kt d",
                                                      d=HD))
            return vd

        dec_state = {"next": 0, "tiles": {}}

        def decode_emit_one():
            s = dec_state["next"]
            if s >= DECODE:
                return False
            dec_state["next"] += 1
            ks_, vs_ = dec_state.setdefault("kd", {}), dec_state.setdefault(
                "vd", {})
            if s == 0:
                ks_[0] = prefetch_kd(0)
                ks_[1] = prefetch_kd(1)
                vs_[0] = prefetch_vd(0)
            if s + 2 < DECODE:
                ks_[s + 2] = prefetch_kd(s + 2)
            if s + 1 < DECODE:
                vs_[s + 1] = prefetch_vd(s + 1)
            kd, vd = ks_.pop(s), vs_.pop(s)

            stp = psBig.tile([P, 72], f32, tag="big", name=f"stp{s}")
            for kt in range(NKT_D):
                nc.tensor.matmul(
                    stp[:, kt * QH:(kt + 1) * QH],
                    lhsT=kd[:, kt * P:(kt + 1) * P],
                    rhs=qdec_sb[:, s * QH:(s + 1) * QH],
                    start=True, stop=True, skip_group_check=True)
            nc.tensor.matmul(
                stp[0:1, 64:68], lhsT=kT_dec[:, s:s + 1],
                rhs=qdec_sb[:, s * QH:(s + 1) * QH],
                start=True, stop=True, skip_group_check=True)
            pt = p3.tile([P, 72], bf, tag="ptd")
            nc.scalar.activation(pt[:, :64], stp[:, :64], AF.Exp, scale=SCALE)
            nc.scalar.activation(pt[0:1, 64:68], stp[0:1, 64:68], AF.Exp,
                                 scale=SCALE)

            vrow = p3.tile([1, HD], bf, tag="vrow")
            nc.gpsimd.dma_start(out=vrow[:], in_=vdt[s:s + 1, :])
            # PV transposed: out [128 d, 4 h] so each matmul is 4 rows
            ov = psBig.tile([P, QH], f32, tag="big", name=f"ov{s}")
            ovs = psSm.tile([1, QH], f32, tag="sm", name=f"ovs{s}")
            for kt in range(NKT_D):
                nc.tensor.matmul(
                    ov[:], lhsT=vd[:, kt, :],
                    rhs=pt[:, kt * QH:(kt + 1) * QH],
                    start=(kt == 0), stop=False, skip_group_check=True)
                nc.tensor.matmul(
                    ovs[:], lhsT=ones_sb[:],
                    rhs=pt[:, kt * QH:(kt + 1) * QH],
                    start=(kt == 0), stop=False, skip_group_check=True)
            nc.tensor.matmul(ov[:], lhsT=vrow[:],
                             rhs=pt[0:1, 64:68], start=False, stop=True,
                             skip_group_check=True)
            nc.tensor.matmul(ovs[:], lhsT=ones_sb[0:1, :],
                             rhs=pt[0:1, 64:68], start=False, stop=True,
                             skip_group_check=True)
            # reciprocal row, then PE-broadcast it down all 128 partitions
            rrow = p3.tile([1, QH], bf, tag="rrow")
            with nc.allow_low_precision(reason="bf16 softmax denom"):
                nc.vector.reciprocal(rrow[:], ovs[:])
            rbc = psSm.tile([P, QH], f32, tag="sm", name=f"rbc{s}")
            nc.tensor.matmul(rbc[:], lhsT=ones_row[:], rhs=rrow[:],
                             start=True, stop=True, skip_group_check=True)
            rbs = p3.tile([P, QH], bf, tag="rbs")
            nc.vector.tensor_copy(out=rbs[:], in_=rbc[:])
            nc.vector.tensor_tensor(attn_dec[:, :, s], ov[:], rbs[:],
                                    ALU.mult)
            if s == DECODE - 1:
                ph4_queue.append(
                    (98, phase4_units(DOFF, 0, DECODE, attn_dec)))
            return True

        def enqueue_ph4(idx, si, t0, L):
            s0, _ = SEQ_BOUNDS[si]
            slot = attn_A if idx % 2 == 0 else attn_B
            ph4_queue.append((idx, phase4_units(s0 + t0, t0, L, slot)))

        # ---------------- phase 2 window ----------------
        def window(idx, si):
            s0, L = SEQ_BOUNDS[si]
            attn_sb = attn_A if idx % 2 == 0 else attn_B
            B = L // P
            debt = [0.0]

            def fill(ns):
                debt[0] += ns
                while debt[0] >= 853.0 and ph4_queue:
                    try:
                        next(ph4_queue[0][1])
                        debt[0] -= 853.0
                    except StopIteration:
                        ph4_queue.pop(0)

            prev = {"pos": None, "Q0": 0, "obufs": None}

            def emit_norm():
                # previous block's softmax normalization (DVE only)
                obufs = []
                for h in range(QH):
                    rr = p3.tile([P, 1], f32, tag="rr")
                    nc.vector.reciprocal(rr[:], prev["pos"][h][:, HD:HD + 1])
                    obuf = p4.tile([P, P], bf, tag="obuf",
                                   name=f"ob{si}_{prev['Q0']}_{h}")
                    nc.vector.tensor_scalar_mul(
                        obuf[:], prev["pos"][h][:, :HD], rr[:])
                    obufs.append(obuf)
                prev["obufs"] = obufs
                prev["pos"] = None

            def emit_transposes():
                Q0p = prev["Q0"]
                for h in range(QH):
                    pst = psSm.tile([P, P], bf, tag="sm")
                    nc.tensor.transpose(pst[:], prev["obufs"][h][:], ident[:])
                    if h % 2 == 0:
                        nc.vector.tensor_copy(
                            out=attn_sb[:, h, Q0p:Q0p + P], in_=pst[:])
                    else:
                        nc.scalar.activation(
                            attn_sb[:, h, Q0p:Q0p + P], pst[:], AF.Copy)
                prev["obufs"] = None
                if (Q0p // P + 1) % 4 == 0:
                    enqueue_ph4(idx, si, Q0p - 3 * P, 512)

            for qb in range(B):
                Q0 = qb * P
                pos = [psPo.tile([P, HD + 1], f32, tag=f"po{h}",
                                 name=f"po{h}_{si}_{qb}") for h in range(QH)]
                q4 = q_sb[:, :, Q0:Q0 + P]
                pending = []   # (pch, c) with exp done, PV not yet emitted

                def emit_pv(pch, c, qb=qb, pos=pos):
                    for h in range(QH):
                        nc.tensor.matmul(
                            pos[h][:], lhsT=pch[:, h, :],
                            rhs=v_nat[:, c, :], start=(c == 0),
                            stop=(c == qb), skip_group_check=True)

                for c in range(qb + 1):
                    sps = psBig.tile([P, 512], f32, tag="big")
                    nc.tensor.matmul(sps[:], lhsT=kT_sb[:, c * P:(c + 1) * P],
                                     rhs=q4, start=True, stop=True,
                                     skip_group_check=True)
                    pch = p4.tile([P, QH, P], bf, tag="pch")
                    nc.scalar.activation(pch[:], sps[:], AF.Exp, scale=SCALE)
                    if c == qb:
                        nc.gpsimd.affine_select(
                            out=pch[:], in_=pch[:], compare_op=ALU.is_ge,
                            fill=0.0, base=0, channel_multiplier=-1,
                            pattern=[[0, QH], [1, P]])
                    if c == 0 and prev["pos"] is not None:
                        emit_norm()
                    if len(pending) >= 2:
                        emit_pv(*pending.pop(0))
                    pending.append((pch, c))
                    if c == 1 and prev["obufs"] is not None:
                        emit_transposes()
                    fill(450.0)
                while pending:
                    emit_pv(*pending.pop(0))
                if prev["obufs"] is not None:   # block had < 2 chunks
                    emit_transposes()
                prev["pos"] = pos
                prev["Q0"] = Q0
                decode_emit_one()
            # final block: norm + transposes at window end
            emit_norm()
            emit_transposes()

        # ---------------- program ----------------
        # ones column of v_nat (rewritten per seq by transposes on :HD only)
        nc.vector.memset(v_nat[:, :, HD:HD + 1], 1.0)

        # seq0 tile 0 first (k/v groups run on wk/wv alone), weight DMAs
        # threaded between so the shared DMA path never starves PE
        qd_tmp = p1.tile([P, QH, DECODE], bf, tag="qd_tmp")
        phase1_tile(0, TW, kT_sb, 0, v_nat, q_sb, 0,
                    wq_dma=(wqT_r, 2), split_ht=4)
        phase1_tile(DOFF, DECODE, kT_dec, 0, vdt, qd_tmp, 0, dec=True)
        for h in range(QH):
            nc.gpsimd.dma_start(out=qdec_r[:, :, h], in_=qd_tmp[:, h, :])

        first_tile_done = True
        for idx, si in enumerate([0, 1, 3, 2]):
            s0, L = SEQ_BOUNDS[si]
            for lt in range(L // TW):
                if si == 0 and lt == 0:
                    continue
                phase1_tile(s0 + lt * TW, TW, kT_sb, lt * TW, v_nat,
                            q_sb, lt * TW)
                decode_emit_one()
                if lt == 1 and idx == 0:
                    nc.sync.dma_start(wo_sb[:], woT_r[:])
            # phase4 of the seq that used this window's attn slot last
            # must be fully drained before this window's first attn write
            drain_older_than(idx - 1)
            window(idx, si)

        ph4_in_window[0] = False
        while ph4_queue:
            pull_ph4(64)
        ph4_in_window[0] = True

        # remaining decode units (assembly handled inside the last unit)
        while decode_emit_one():
            pass
        while ph4_queue:
            pull_ph4(64)

    nc.compile()
    return nc


_NC = None


def _get_program():
    global _NC
    if _NC is None:
        _NC = build_program()
    return _NC


def _rope_tables():
    inv_freq = 1.0 / (10000.0 ** (np.arange(0, HD, 2, dtype=np.float32) / HD))
    pos_q = np.concatenate(
        [np.arange(L, dtype=np.float32) for L in PREFILLS]
        + [np.full(DECODE, float(PAST), np.float32)])                 # [T]
    ang_q = np.outer(inv_freq, pos_q)                                 # [64, T]
    qcos = np.concatenate([np.cos(ang_q), np.cos(ang_q)], axis=0)
    qsin = np.concatenate([-np.sin(ang_q), np.sin(ang_q)], axis=0)
    return qcos.astype(BF16), qsin.astype(BF16)


def _rope_cache(k):
    # k: [DECODE, PAST, HD] float32; positions 0..PAST-1
    inv_freq = 1.0 / (10000.0 ** (np.arange(0, HD, 2, dtype=np.float32) / HD))
    ang = np.outer(np.arange(PAST, dtype=np.float32), inv_freq)  # [PAST, 64]
    cos = np.concatenate([np.cos(ang), np.cos(ang)], axis=1)     # [PAST, 128]
    sin = np.concatenate([np.sin(ang), np.sin(ang)], axis=1)
    rot = np.concatenate([-k[..., HD // 2:], k[..., :HD // 2]], axis=-1)
    return k * cos[None] + rot * sin[None]


def make_in_maps(hidden_states, wq, wk, wv, wo, kv_cache_k, kv_cache_v):
    hidden_states = np.asarray(hidden_states)
    wq, wk, wv, wo = (np.asarray(a) for a in (wq, wk, wv, wo))
    kv_cache_k, kv_cache_v = np.asarray(kv_cache_k), np.asarray(kv_cache_v)

    hT = np.ascontiguousarray(hidden_states.astype(BF16).T)      # [4096, T]
    qcos, qsin = _rope_tables()
    in_maps = []
    for c in range(NCORES):
        wqT = np.ascontiguousarray(wq[c * ADIM:(c + 1) * ADIM, :]
                                   .astype(BF16).T)
        wkT = np.ascontiguousarray(wk[c * HD:(c + 1) * HD, :].astype(BF16).T)
        wvT = np.ascontiguousarray(wv[c * HD:(c + 1) * HD, :].astype(BF16).T)
        woT = np.ascontiguousarray(wo[:, c * ADIM:(c + 1) * ADIM]
                                   .astype(BF16).T)
        kc = _rope_cache(kv_cache_k[:, :, c, :].astype(np.float32))
        kTc = np.ascontiguousarray(
            kc.astype(BF16).transpose(0, 2, 1))                  # [32,128,2048]
        vcc = kv_cache_v[:, :, c, :].astype(BF16)                # [32,2048,128]
        vcp = np.ascontiguousarray(
            vcc.reshape(DECODE, NKT_D, P, HD).transpose(0, 2, 1, 3)
            .reshape(DECODE, P, PAST))
        in_maps.append({
            "hT": hT, "wqT": wqT, "wkT": wkT, "wvT": wvT, "woT": woT,
            "kTc": kTc, "vcp": vcp, "qcos": qcos, "qsin": qsin,
        })
    return in_maps


def combine_outputs(results):
    acc = np.zeros((HIDDEN, T), np.float32)
    for c in range(NCORES):
        acc += results[c]["outT"].astype(np.float32)
    return np.ascontiguousarray(acc.T)


def kernel(hidden_states, wq, wk, wv, wo, kv_cache_k, kv_cache_v):
    from concourse.bass_utils import run_bass_kernel_spmd

    nc = _get_program()
    in_maps = make_in_maps(hidden_states, wq, wk, wv, wo,
                           kv_cache_k, kv_cache_v)
    res = run_bass_kernel_spmd(nc, in_maps, core_ids=list(range(NCORES)))
    return combine_outputs(res.results)


# revision 30
# speedup vs baseline: 1.0026x; 1.0024x over previous
"""Trainium2 Bass kernel for nn_LlamaAttention_61495341744411.

Sharding: tensor-parallel over heads across 8 NeuronCores.
  core c: q heads [4c, 4c+4), kv head c, wo cols [512c, 512c+512).
  Each core computes a full-token partial of out^T; host sums partials.

Device program per core (single SPMD Bass program, bf16 matmuls):
  P1: q/k/v projections from hidden^T (host-transposed), RoPE fused.
      q and k stay resident in SBUF (no DRAM spill).
  P2: causal prefill attention computed score-transposed: S^T chunks
      [keys, 4h*128q] feed exp directly into the PV matmul as lhsT —
      no P transposes, no PSUM->SBUF P copies. Softmax denominators
      come from a ones-column appended to V.
  P3: paged decode attention, KV cache pre-RoPE'd and packed on host,
      interleaved across the whole timeline (32 single-token seqs).
  P4: o_proj partial out^T = woT.T @ attn^T, emitted as PE filler
      inside the P2 windows (which are Activation-bound) + tail.

Engine discipline: every op is routed to a fixed engine so no in-order
queue ever holds an op whose dependency resolves far in the future.
"""
import sys

if "/opt/trn_rl_repo" not in sys.path:
    sys.path.insert(0, "/opt/trn_rl_repo")

import numpy as np
import ml_dtypes

BF16 = ml_dtypes.bfloat16

PREFILLS = [1024, 1536, 2048, 512]
DOFF = sum(PREFILLS)            # 5120
DECODE = 32
PAST = 2048
HIDDEN = 4096
NQ, NKV, HD = 32, 8, 128
G = NQ // NKV                   # 4
T = DOFF + DECODE               # 5152
SCALE = 1.0 / float(np.sqrt(HD))
NCORES = 8
QH = NQ // NCORES               # 4 q heads per core
ADIM = QH * HD                  # 512
KS = HIDDEN // 128              # 32 contraction subtiles
P = 128
TW = 256                        # projection t-tile width
NKT_D = PAST // P               # 16 decode cache k-tiles
NB = QH + 1                     # rope'd projection rows: [k, q0..q3]
LMAX = max(PREFILLS)

SEQ_BOUNDS = []
_off = 0
for _L in PREFILLS:
    SEQ_BOUNDS.append((_off, _L))
    _off += _L


def build_program():
    import concourse.mybir as mybir
    import concourse.tile as tile
    from concourse import bacc
    from concourse.masks import make_identity
    from contextlib import ExitStack

    dt = mybir.dt
    AF = mybir.ActivationFunctionType
    ALU = mybir.AluOpType
    f32 = dt.float32
    bf = dt.bfloat16

    nc = bacc.Bacc(None, target_bir_lowering=False, debug=False)

    hT = nc.dram_tensor("hT", [HIDDEN, T], bf, kind="ExternalInput")
    wqT = nc.dram_tensor("wqT", [HIDDEN, ADIM], bf, kind="ExternalInput")
    wkT = nc.dram_tensor("wkT", [HIDDEN, HD], bf, kind="ExternalInput")
    wvT = nc.dram_tensor("wvT", [HIDDEN, HD], bf, kind="ExternalInput")
    woT = nc.dram_tensor("woT", [ADIM, HIDDEN], bf, kind="ExternalInput")
    kTc = nc.dram_tensor("kTc", [DECODE, HD, PAST], bf, kind="ExternalInput")
    vcp = nc.dram_tensor("vcp", [DECODE, P, PAST], bf, kind="ExternalInput")
    qcos = nc.dram_tensor("qcos", [HD, T], bf, kind="ExternalInput")
    qsin = nc.dram_tensor("qsin", [HD, T], bf, kind="ExternalInput")
    outT = nc.dram_tensor("outT", [HIDDEN, T], bf, kind="ExternalOutput")
    import os as _os
    DBG = bool(_os.environ.get("KBDBG"))
    if DBG:
        dbg_q = nc.dram_tensor("dbg_q", [P, QH, 1024], bf, kind="ExternalOutput")
        dbg_k = nc.dram_tensor("dbg_k", [P, 1024], bf, kind="ExternalOutput")
        dbg_v = nc.dram_tensor("dbg_v", [P, 8, HD + 1], bf, kind="ExternalOutput")
        dbg_a = nc.dram_tensor("dbg_a", [P, QH, 1024], bf, kind="ExternalOutput")

    hT_r = hT.rearrange("(o p) t -> p o t", p=P)        # [128, 32, T]
    wqT_r = wqT.rearrange("(o p) m -> p o m", p=P)      # [128, 32, 512]
    wkT_r = wkT.rearrange("(o p) m -> p o m", p=P)      # [128, 32, 128]
    wvT_r = wvT.rearrange("(o p) m -> p o m", p=P)
    woT_r = woT.rearrange("(o p) m -> p o m", p=P)      # [128, 4, 4096]
    outT_r = outT.rearrange("(o p) t -> p o t", p=P)    # [128, 32, T]

    with ExitStack() as ctx:
        tc = ctx.enter_context(tile.TileContext(nc))
        p1 = ctx.enter_context(tc.tile_pool(name="p1", bufs=1))
        p2 = ctx.enter_context(tc.tile_pool(name="p2", bufs=2))
        p3 = ctx.enter_context(tc.tile_pool(name="p3", bufs=3))
        p4 = ctx.enter_context(tc.tile_pool(name="p4", bufs=4))
        pDecK = ctx.enter_context(tc.tile_pool(name="pDecK", bufs=3))
        pDecV = ctx.enter_context(tc.tile_pool(name="pDecV", bufs=2))
        psBig = ctx.enter_context(tc.tile_pool(name="psBig", bufs=3,
                                               space="PSUM"))
        psPo = ctx.enter_context(tc.tile_pool(name="psPo", bufs=1,
                                              space="PSUM"))
        psSm = ctx.enter_context(tc.tile_pool(name="psSm", bufs=1,
                                              space="PSUM"))

        ident = p1.tile([P, P], bf, tag="ident")
        make_identity(nc, ident)
        ones_sb = p1.tile([P, 1], bf, tag="ones")
        nc.vector.memset(ones_sb[:], 1.0)
        ones_row = p1.tile([1, P], bf, tag="ones_row")
        nc.vector.memset(ones_row[:], 1.0)

        # resident weights (wq/wo DMAs deferred below so the first
        # ht tiles aren't starved on the shared DMA path)
        wk_sb = p1.tile([P, KS, HD], bf, tag="wk")
        nc.sync.dma_start(wk_sb[:], wkT_r[:])
        wv_sb = p1.tile([P, KS, HD], bf, tag="wv")
        wq_sb = p1.tile([P, KS, ADIM], bf, tag="wq")
        wo_sb = p1.tile([P, QH, HIDDEN], bf, tag="wo")

        # resident per-seq activations (single slot, reused across seqs)
        q_sb = p1.tile([P, QH, LMAX], bf, tag="q_sb")
        attn_A = p1.tile([P, QH, 1024], bf, tag="attn_A")
        attn_B = p1.tile([P, QH, LMAX], bf, tag="attn_B")
        kT_sb = p1.tile([P, LMAX], bf, tag="kT_sb")
        v_nat = p1.tile([P, LMAX // P, HD + 1], bf, tag="v_nat")

        # decode residents
        kT_dec = p1.tile([P, DECODE], bf, tag="kTdec")
        vdt = p1.tile([DECODE, HD], bf, tag="vdt")
        qdec_sb = p1.tile([P, P], bf, tag="qdec")      # cols (s,h)
        odec_sb = p1.tile([P, HD], bf, tag="odec")     # rows (s,h)
        attn_dec = p1.tile([P, QH, DECODE], bf, tag="attn_dec")

        qdec_r = qdec_sb.rearrange("p (s h) -> p s h", h=QH)

        # ---------------- phase 1: projections + rope ----------------
        def phase1_tile(t0, W, kT_dst, kcol0, v_dst, q_dst, qcol0,
                        dec=False, wq_dma=None, split_ht=1):
            H2 = KS // 2
            hta = p2.tile([P, H2, TW], bf, tag="hta")
            htb = p2.tile([P, H2, TW], bf, tag="htb")
            for half, ht_t in ((0, hta), (1, htb)):
                for i in range(split_ht):
                    sl = slice(i * H2 // split_ht, (i + 1) * H2 // split_ht)
                    sl2 = slice(half * H2 + i * H2 // split_ht,
                                half * H2 + (i + 1) * H2 // split_ht)
                    nc.sync.dma_start(ht_t[:, sl, :W], hT_r[:, sl2, t0:t0 + W])
            if split_ht > 1:
                nc.sync.dma_start(wv_sb[:], wvT_r[:])
            ct = p2.tile([P, TW], bf, tag="cos")
            st = p2.tile([P, TW], bf, tag="sin")
            nc.gpsimd.dma_start(ct[:, :W], qcos[:, t0:t0 + W])
            nc.gpsimd.dma_start(st[:, :W], qsin[:, t0:t0 + W])

            xq = p2.tile([P, NB, TW], bf, tag="xq")

            def group(w_ap, sink):
                ps = psBig.tile([P, 512], f32, tag="big")
                for ks in range(KS):
                    src = hta if ks < KS // 2 else htb
                    nc.tensor.matmul(
                        ps[:, :W], lhsT=w_ap[:, ks, :],
                        rhs=src[:, ks % (KS // 2), :W],
                        start=(ks == 0), stop=(ks == KS - 1),
                        skip_group_check=True)
                sink(ps)

            # k group -> xq row 0
            group(wk_sb, lambda ps: nc.scalar.activation(
                xq[:, 0, :W], ps[:, :W], AF.Copy))

            # v group -> transposes into v_dst
            def v_sink(ps):
                vt = p3.tile([P, TW], bf, tag="vt")
                nc.vector.tensor_copy(out=vt[:, :W], in_=ps[:, :W])
                if not dec:
                    for j in range(W // P):
                        pst = psSm.tile([P, P], bf, tag="sm")
                        nc.tensor.transpose(
                            pst[:], vt[:, j * P:(j + 1) * P], ident[:])
                        nc.vector.tensor_copy(
                            out=v_dst[:, kcol0 // P + j, :HD], in_=pst[:])
                else:
                    pst = psSm.tile([P, P], bf, tag="sm")
                    nc.tensor.transpose(pst[:DECODE, :], vt[:, :W], ident[:])
                    nc.vector.tensor_copy(out=v_dst[:], in_=pst[:DECODE, :])
            group(wv_sb, v_sink)

            if wq_dma is not None:
                wsrc, nchunks = wq_dma
                half = ADIM // nchunks
                for i in range(nchunks):
                    nc.sync.dma_start(
                        wq_sb[:, :, i * half:(i + 1) * half],
                        wsrc[:, :, i * half:(i + 1) * half])

            # q groups -> xq rows 1..4
            for m in range(QH):
                group(wq_sb[:, :, m * P:(m + 1) * P],
                      lambda ps, m=m: nc.scalar.activation(
                          xq[:, 1 + m, :W], ps[:, :W], AF.Copy))

            # rope on xq rows (k + 4 q)
            rotq = p2.tile([P, NB, TW], bf, tag="rotq")
            nc.gpsimd.dma_start(out=rotq[0:64, :, :W], in_=xq[64:128, :, :W])
            nc.gpsimd.dma_start(out=rotq[64:128, :, :W], in_=xq[0:64, :, :W])
            ct_b = ct[:, None, :W].to_broadcast((P, NB, W))
            st_b = st[:, None, :W].to_broadcast((P, NB, W))
            nc.vector.tensor_tensor(xq[:, :, :W], xq[:, :, :W], ct_b,
                                    ALU.mult)
            nc.vector.tensor_tensor(rotq[:, :, :W], rotq[:, :, :W], st_b,
                                    ALU.mult)
            nc.vector.tensor_tensor(
                kT_dst[:, kcol0:kcol0 + W], xq[:, 0, :W], rotq[:, 0, :W],
                ALU.add)
            nc.vector.tensor_tensor(
                q_dst[:, :, qcol0:qcol0 + W], xq[:, 1:NB, :W],
                rotq[:, 1:NB, :W], ALU.add)

        # ---------------- phase 4 unit generator ----------------
        NOMB = 2
        omb = {"buf": None, "fill": 0}
        ph4_in_window = [True]

        def phase4_units(out0, src0, L, src):
            """Yield once per m-block (4 matmuls + copy [+dma])."""
            for t0 in range(0, L, 512):
                W = min(512, L - t0)
                for mg in range(KS):
                    ps = psBig.tile([P, 512], f32, tag="big")
                    for ks in range(QH):
                        nc.tensor.matmul(
                            ps[:, :W], lhsT=wo_sb[:, ks, mg * P:(mg + 1) * P],
                            rhs=src[:, ks, src0 + t0:src0 + t0 + W],
                            start=(ks == 0), stop=(ks == QH - 1),
                            skip_group_check=True)
                    if omb["buf"] is None:
                        omb["n"] = omb.get("n", 0) + 1
                        omb["buf"] = p2.tile([P, NOMB, 512], bf, tag="omb",
                                             name=f"omb{omb['n']}")
                        omb["fill"] = 0
                    ob = omb["buf"]
                    slot = omb["fill"]
                    if ph4_in_window[0] or mg % 2 == 0:
                        nc.vector.tensor_copy(out=ob[:, slot, :W],
                                              in_=ps[:, :W])
                    else:
                        nc.scalar.activation(ob[:, slot, :W], ps[:, :W],
                                             AF.Copy)
                    omb["fill"] += 1
                    if omb["fill"] == NOMB:
                        nc.sync.dma_start(
                            outT_r[:, mg - NOMB + 1:mg + 1,
                                   out0 + t0:out0 + t0 + W],
                            ob[:, :, :W])
                        omb["buf"] = None
                    yield

        ph4_queue = []

        def pull_ph4(n):
            for _ in range(n):
                if not ph4_queue:
                    return
                try:
                    next(ph4_queue[0][1])
                except StopIteration:
                    ph4_queue.pop(0)

        def drain_older_than(idx):
            while ph4_queue and ph4_queue[0][0] < idx:
                try:
                    next(ph4_queue[0][1])
                except StopIteration:
                    ph4_queue.pop(0)

        # ---------------- decode units ----------------
        def prefetch_kd(s):
            kd = pDecK.tile([P, PAST], bf, tag="kd", name=f"kd{s}")
            nc.sync.dma_start(kd[:], kTc[s])
            return kd

        def prefetch_vd(s):
            vd = pDecV.tile([P, NKT_D, HD], bf, tag="vd", name=f"vd{s}")
            nc.sync.dma_start(vd[:], vcp[s].rearrange("p (kt d) -> p kt d",
                                                      d=HD))
            return vd

        dec_state = {"next": 0, "tiles": {}}

        def decode_emit_one():
            s = dec_state["next"]
            if s >= DECODE:
                return False
            dec_state["next"] += 1
            ks_, vs_ = dec_state.setdefault("kd", {}), dec_state.setdefault(
                "vd", {})
            if s == 0:
                ks_[0] = prefetch_kd(0)
                ks_[1] = prefetch_kd(1)
                vs_[0] = prefetch_vd(0)
            if s + 2 < DECODE:
                ks_[s + 2] = prefetch_kd(s + 2)
            if s + 1 < DECODE:
                vs_[s + 1] = prefetch_vd(s + 1)
            kd, vd = ks_.pop(s), vs_.pop(s)

            stp = psBig.tile([P, 72], f32, tag="big", name=f"stp{s}")
            for kt in range(NKT_D):
                nc.tensor.matmul(
                    stp[:, kt * QH:(kt + 1) * QH],
                    lhsT=kd[:, kt * P:(kt + 1) * P],
                    rhs=qdec_sb[:, s * QH:(s + 1) * QH],
                    start=True, stop=True, skip_group_check=True)
            nc.tensor.matmul(
                stp[0:1, 64:68], lhsT=kT_dec[:, s:s + 1],
                rhs=qdec_sb[:, s * QH:(s + 1) * QH],
                start=True, stop=True, skip_group_check=True)
            pt = p3.tile([P, 72], bf, tag="ptd")
            nc.scalar.activation(pt[:, :64], stp[:, :64], AF.Exp, scale=SCALE)
            nc.scalar.activation(pt[0:1, 64:68], stp[0:1, 64:68], AF.Exp,
                                 scale=SCALE)

            vrow = p3.tile([1, HD], bf, tag="vrow")
            nc.gpsimd.dma_start(out=vrow[:], in_=vdt[s:s + 1, :])
            # PV transposed: out [128 d, 4 h] so each matmul is 4 rows
            ov = psBig.tile([P, QH], f32, tag="big", name=f"ov{s}")
            ovs = psSm.tile([1, QH], f32, tag="sm", name=f"ovs{s}")
            for kt in range(NKT_D):
                nc.tensor.matmul(
                    ov[:], lhsT=vd[:, kt, :],
                    rhs=pt[:, kt * QH:(kt + 1) * QH],
                    start=(kt == 0), stop=False, skip_group_check=True)
                nc.tensor.matmul(
                    ovs[:], lhsT=ones_sb[:],
                    rhs=pt[:, kt * QH:(kt + 1) * QH],
                    start=(kt == 0), stop=False, skip_group_check=True)
            nc.tensor.matmul(ov[:], lhsT=vrow[:],
                             rhs=pt[0:1, 64:68], start=False, stop=True,
                             skip_group_check=True)
            nc.tensor.matmul(ovs[:], lhsT=ones_sb[0:1, :],
                             rhs=pt[0:1, 64:68], start=False, stop=True,
                             skip_group_check=True)
            # reciprocal row, then PE-broadcast it down all 128 partitions
            rrow = p3.tile([1, QH], bf, tag="rrow")
            with nc.allow_low_precision(reason="bf16 softmax denom"):
                nc.vector.reciprocal(rrow[:], ovs[:])
            rbc = psSm.tile([P, QH], f32, tag="sm", name=f"rbc{s}")
            nc.tensor.matmul(rbc[:], lhsT=ones_row[:], rhs=rrow[:],
                             start=True, stop=True, skip_group_check=True)
            rbs = p3.tile([P, QH], bf, tag="rbs")
            nc.vector.tensor_copy(out=rbs[:], in_=rbc[:])
            nc.vector.tensor_tensor(attn_dec[:, :, s], ov[:], rbs[:],
                                    ALU.mult)
            if s == DECODE - 1:
                ph4_queue.append(
                    (98, phase4_units(DOFF, 0, DECODE, attn_dec)))
            return True

        def enqueue_ph4(idx, si, t0, L):
            s0, _ = SEQ_BOUNDS[si]
            slot = attn_A if idx % 2 == 0 else attn_B
            ph4_queue.append((idx, phase4_units(s0 + t0, t0, L, slot)))

        # ---------------- phase 2 window ----------------
        def window(idx, si):
            s0, L = SEQ_BOUNDS[si]
            attn_sb = attn_A if idx % 2 == 0 else attn_B
            B = L // P
            debt = [0.0]

            def fill(ns):
                debt[0] += ns
                while debt[0] >= 853.0 and ph4_queue:
                    try:
                        next(ph4_queue[0][1])
                        debt[0] -= 853.0
                    except StopIteration:
                        ph4_queue.pop(0)

            prev = {"pos": None, "Q0": 0, "obufs": None}

            def emit_norm():
                # previous block's softmax normalization (DVE only)
                obufs = []
                for h in range(QH):
                    rr = p3.tile([P, 1], f32, tag="rr")
                    nc.vector.reciprocal(rr[:], prev["pos"][h][:, HD:HD + 1])
                    obuf = p4.tile([P, P], bf, tag="obuf",
                                   name=f"ob{si}_{prev['Q0']}_{h}")
                    nc.vector.tensor_scalar_mul(
                        obuf[:], prev["pos"][h][:, :HD], rr[:])
                    obufs.append(obuf)
                prev["obufs"] = obufs
                prev["pos"] = None

            def emit_transposes():
                Q0p = prev["Q0"]
                for h in range(QH):
                    pst = psSm.tile([P, P], bf, tag="sm")
                    nc.tensor.transpose(pst[:], prev["obufs"][h][:], ident[:])
                    if h % 2 == 0:
                        nc.vector.tensor_copy(
                            out=attn_sb[:, h, Q0p:Q0p + P], in_=pst[:])
                    else:
                        nc.scalar.activation(
                            attn_sb[:, h, Q0p:Q0p + P], pst[:], AF.Copy)
                prev["obufs"] = None
                if (Q0p // P + 1) % 4 == 0:
                    enqueue_ph4(idx, si, Q0p - 3 * P, 512)

            for qb in range(B):
                Q0 = qb * P
                pos = [psPo.tile([P, HD + 1], f32, tag=f"po{h}",
                                 name=f"po{h}_{si}_{qb}") for h in range(QH)]
                q4 = q_sb[:, :, Q0:Q0 + P]
                pending = []   # (pch, c) with exp done, PV not yet emitted

                def emit_pv(pch, c, qb=qb, pos=pos):
                    for h in range(QH):
                        nc.tensor.matmul(
                            pos[h][:], lhsT=pch[:, h, :],
                            rhs=v_nat[:, c, :], start=(c == 0),
                            stop=(c == qb), skip_group_check=True)

                for c in range(qb + 1):
                    sps = psBig.tile([P, 512], f32, tag="big")
                    nc.tensor.matmul(sps[:], lhsT=kT_sb[:, c * P:(c + 1) * P],
                                     rhs=q4, start=True, stop=True,
                                     skip_group_check=True)
                    pch = p4.tile([P, QH, P], bf, tag="pch")
                    nc.scalar.activation(pch[:], sps[:], AF.Exp, scale=SCALE)
                    if c == qb:
                        nc.gpsimd.affine_select(
                            out=pch[:], in_=pch[:], compare_op=ALU.is_ge,
                            fill=0.0, base=0, channel_multiplier=-1,
                            pattern=[[0, QH], [1, P]])
                    if c == 0 and prev["pos"] is not None:
                        emit_norm()
                    if len(pending) >= 2:
                        emit_pv(*pending.pop(0))
                    pending.append((pch, c))
                    if c == 1 and prev["obufs"] is not None:
                        emit_transposes()
                    fill(450.0)
                while pending:
                    emit_pv(*pending.pop(0))
                if prev["obufs"] is not None:   # block had < 2 chunks
                    emit_transposes()
                prev["pos"] = pos
                prev["Q0"] = Q0
                decode_emit_one()
            # final block: norm + transposes at window end
            emit_norm()
            emit_transposes()

        # ---------------- program ----------------
        # ones column of v_nat (rewritten per seq by transposes on :HD only)
        nc.vector.memset(v_nat[:, :, HD:HD + 1], 1.0)

        # seq0 tile 0 first (k/v groups run on wk/wv alone), weight DMAs
        # threaded between so the shared DMA path never starves PE
        qd_tmp = p1.tile([P, QH, DECODE], bf, tag="qd_tmp")
        phase1_tile(0, TW, kT_sb, 0, v_nat, q_sb, 0,
                    wq_dma=(wqT_r, 2), split_ht=4)
        phase1_tile(DOFF, DECODE, kT_dec, 0, vdt, qd_tmp, 0, dec=True)
        for h in range(QH):
            nc.gpsimd.dma_start(out=qdec_r[:, :, h], in_=qd_tmp[:, h, :])

        first_tile_done = True
        for idx, si in enumerate([0, 1, 3, 2]):
            s0, L = SEQ_BOUNDS[si]
            for lt in range(L // TW):
                if si == 0 and lt == 0:
                    continue
                phase1_tile(s0 + lt * TW, TW, kT_sb, lt * TW, v_nat,
                            q_sb, lt * TW)
                decode_emit_one()
                if lt == 1 and idx == 0:
                    nc.sync.dma_start(wo_sb[:], woT_r[:])
            # phase4 of the seq that used this window's attn slot last
            # must be fully drained before this window's first attn write
            ph4_in_window[0] = False
            drain_older_than(idx - 1)
            ph4_in_window[0] = True
            window(idx, si)

        ph4_in_window[0] = False
        while ph4_queue:
            pull_ph4(64)
        ph4_in_window[0] = True

        # remaining decode units (assembly handled inside the last unit)
        while decode_emit_one():
            pass
        while ph4_queue:
            pull_ph4(64)

    nc.compile()
    return nc


_NC = None


def _get_program():
    global _NC
    if _NC is None:
        _NC = build_program()
    return _NC


def _rope_tables():
    inv_freq = 1.0 / (10000.0 ** (np.arange(0, HD, 2, dtype=np.float32) / HD))
    pos_q = np.concatenate(
        [np.arange(L, dtype=np.float32) for L in PREFILLS]
        + [np.full(DECODE, float(PAST), np.float32)])                 # [T]
    ang_q = np.outer(inv_freq, pos_q)                                 # [64, T]
    qcos = np.concatenate([np.cos(ang_q), np.cos(ang_q)], axis=0)
    qsin = np.concatenate([-np.sin(ang_q), np.sin(ang_q)], axis=0)
    return qcos.astype(BF16), qsin.astype(BF16)


def _rope_cache(k):
    # k: [DECODE, PAST, HD] float32; positions 0..PAST-1
    inv_freq = 1.0 / (10000.0 ** (np.arange(0, HD, 2, dtype=np.float32) / HD))
    ang = np.outer(np.arange(PAST, dtype=np.float32), inv_freq)  # [PAST, 64]
    cos = np.concatenate([np.cos(ang), np.cos(ang)], axis=1)     # [PAST, 128]
    sin = np.concatenate([np.sin(ang), np.sin(ang)], axis=1)
    rot = np.concatenate([-k[..., HD // 2:], k[..., :HD // 2]], axis=-1)
    return k * cos[None] + rot * sin[None]


def make_in_maps(hidden_states, wq, wk, wv, wo, kv_cache_k, kv_cache_v):
    hidden_states = np.asarray(hidden_states)
    wq, wk, wv, wo = (np.asarray(a) for a in (wq, wk, wv, wo))
    kv_cache_k, kv_cache_v = np.asarray(kv_cache_k), np.asarray(kv_cache_v)

    hT = np.ascontiguousarray(hidden_states.astype(BF16).T)      # [4096, T]
    qcos, qsin = _rope_tables()
    in_maps = []
    for c in range(NCORES):
        wqT = np.ascontiguousarray(wq[c * ADIM:(c + 1) * ADIM, :]
                                   .astype(BF16).T)
        wkT = np.ascontiguousarray(wk[c * HD:(c + 1) * HD, :].astype(BF16).T)
        wvT = np.ascontiguousarray(wv[c * HD:(c + 1) * HD, :].astype(BF16).T)
        woT = np.ascontiguousarray(wo[:, c * ADIM:(c + 1) * ADIM]
                                   .astype(BF16).T)
        kc = _rope_cache(kv_cache_k[:, :, c, :].astype(np.float32))
        kTc = np.ascontiguousarray(
            kc.astype(BF16).transpose(0, 2, 1))                  # [32,128,2048]
        vcc = kv_cache_v[:, :, c, :].astype(BF16)                # [32,2048,128]
        vcp = np.ascontiguousarray(
            vcc.reshape(DECODE, NKT_D, P, HD).transpose(0, 2, 1, 3)
            .reshape(DECODE, P, PAST))
        in_maps.append({
            "hT": hT, "wqT": wqT, "wkT": wkT, "wvT": wvT, "woT": woT,
            "kTc": kTc, "vcp": vcp, "qcos": qcos, "qsin": qsin,
        })
    return in_maps


def combine_outputs(results):
    acc = np.zeros((HIDDEN, T), np.float32)
    for c in range(NCORES):
        acc += results[c]["outT"].astype(np.float32)
    return np.ascontiguousarray(acc.T)


def kernel(hidden_states, wq, wk, wv, wo, kv_cache_k, kv_cache_v):
    from concourse.bass_utils import run_bass_kernel_spmd

    nc = _get_program()
    in_maps = make_in_maps(hidden_states, wq, wk, wv, wo,
                           kv_cache_k, kv_cache_v)
    res = run_bass_kernel_spmd(nc, in_maps, core_ids=list(range(NCORES)))
    return combine_outputs(res.results)


# revision 31
# speedup vs baseline: 1.0058x; 1.0032x over previous
"""Trainium2 Bass kernel for nn_LlamaAttention_61495341744411.

Sharding: tensor-parallel over heads across 8 NeuronCores.
  core c: q heads [4c, 4c+4), kv head c, wo cols [512c, 512c+512).
  Each core computes a full-token partial of out^T; host sums partials.

Device program per core (single SPMD Bass program, bf16 matmuls):
  P1: q/k/v projections from hidden^T (host-transposed), RoPE fused.
      q and k stay resident in SBUF (no DRAM spill).
  P2: causal prefill attention computed score-transposed: S^T chunks
      [keys, 4h*128q] feed exp directly into the PV matmul as lhsT —
      no P transposes, no PSUM->SBUF P copies. Softmax denominators
      come from a ones-column appended to V.
  P3: paged decode attention, KV cache pre-RoPE'd and packed on host,
      interleaved across the whole timeline (32 single-token seqs).
  P4: o_proj partial out^T = woT.T @ attn^T, emitted as PE filler
      inside the P2 windows (which are Activation-bound) + tail.

Engine discipline: every op is routed to a fixed engine so no in-order
queue ever holds an op whose dependency resolves far in the future.
"""
import sys

if "/opt/trn_rl_repo" not in sys.path:
    sys.path.insert(0, "/opt/trn_rl_repo")

import numpy as np
import ml_dtypes

BF16 = ml_dtypes.bfloat16

PREFILLS = [1024, 1536, 2048, 512]
DOFF = sum(PREFILLS)            # 5120
DECODE = 32
PAST = 2048
HIDDEN = 4096
NQ, NKV, HD = 32, 8, 128
G = NQ // NKV                   # 4
T = DOFF + DECODE               # 5152
SCALE = 1.0 / float(np.sqrt(HD))
NCORES = 8
QH = NQ // NCORES               # 4 q heads per core
ADIM = QH * HD                  # 512
KS = HIDDEN // 128              # 32 contraction subtiles
P = 128
TW = 256                        # projection t-tile width
NKT_D = PAST // P               # 16 decode cache k-tiles
NB = QH + 1                     # rope'd projection rows: [k, q0..q3]
LMAX = max(PREFILLS)

SEQ_BOUNDS = []
_off = 0
for _L in PREFILLS:
    SEQ_BOUNDS.append((_off, _L))
    _off += _L


def build_program():
    import concourse.mybir as mybir
    import concourse.tile as tile
    from concourse import bacc
    from concourse.masks import make_identity
    from contextlib import ExitStack

    dt = mybir.dt
    AF = mybir.ActivationFunctionType
    ALU = mybir.AluOpType
    f32 = dt.float32
    bf = dt.bfloat16

    nc = bacc.Bacc(None, target_bir_lowering=False, debug=False)

    hT = nc.dram_tensor("hT", [HIDDEN, T], bf, kind="ExternalInput")
    wqT = nc.dram_tensor("wqT", [HIDDEN, ADIM], bf, kind="ExternalInput")
    wkT = nc.dram_tensor("wkT", [HIDDEN, HD], bf, kind="ExternalInput")
    wvT = nc.dram_tensor("wvT", [HIDDEN, HD], bf, kind="ExternalInput")
    woT = nc.dram_tensor("woT", [ADIM, HIDDEN], bf, kind="ExternalInput")
    kTc = nc.dram_tensor("kTc", [DECODE, HD, PAST], bf, kind="ExternalInput")
    vcp = nc.dram_tensor("vcp", [DECODE, P, PAST], bf, kind="ExternalInput")
    qcos = nc.dram_tensor("qcos", [HD, T], bf, kind="ExternalInput")
    qsin = nc.dram_tensor("qsin", [HD, T], bf, kind="ExternalInput")
    outT = nc.dram_tensor("outT", [HIDDEN, T], bf, kind="ExternalOutput")
    import os as _os
    DBG = bool(_os.environ.get("KBDBG"))
    if DBG:
        dbg_q = nc.dram_tensor("dbg_q", [P, QH, 1024], bf, kind="ExternalOutput")
        dbg_k = nc.dram_tensor("dbg_k", [P, 1024], bf, kind="ExternalOutput")
        dbg_v = nc.dram_tensor("dbg_v", [P, 8, HD + 1], bf, kind="ExternalOutput")
        dbg_a = nc.dram_tensor("dbg_a", [P, QH, 1024], bf, kind="ExternalOutput")

    hT_r = hT.rearrange("(o p) t -> p o t", p=P)        # [128, 32, T]
    wqT_r = wqT.rearrange("(o p) m -> p o m", p=P)      # [128, 32, 512]
    wkT_r = wkT.rearrange("(o p) m -> p o m", p=P)      # [128, 32, 128]
    wvT_r = wvT.rearrange("(o p) m -> p o m", p=P)
    woT_r = woT.rearrange("(o p) m -> p o m", p=P)      # [128, 4, 4096]
    outT_r = outT.rearrange("(o p) t -> p o t", p=P)    # [128, 32, T]

    with ExitStack() as ctx:
        tc = ctx.enter_context(tile.TileContext(nc))
        p1 = ctx.enter_context(tc.tile_pool(name="p1", bufs=1))
        p2 = ctx.enter_context(tc.tile_pool(name="p2", bufs=2))
        p3 = ctx.enter_context(tc.tile_pool(name="p3", bufs=3))
        p4 = ctx.enter_context(tc.tile_pool(name="p4", bufs=4))
        pDecK = ctx.enter_context(tc.tile_pool(name="pDecK", bufs=3))
        pDecV = ctx.enter_context(tc.tile_pool(name="pDecV", bufs=2))
        psBig = ctx.enter_context(tc.tile_pool(name="psBig", bufs=3,
                                               space="PSUM"))
        psPo = ctx.enter_context(tc.tile_pool(name="psPo", bufs=1,
                                              space="PSUM"))
        psSm = ctx.enter_context(tc.tile_pool(name="psSm", bufs=1,
                                              space="PSUM"))

        ident = p1.tile([P, P], bf, tag="ident")
        make_identity(nc, ident)
        ones_sb = p1.tile([P, 1], bf, tag="ones")
        nc.vector.memset(ones_sb[:], 1.0)
        ones_row = p1.tile([1, P], bf, tag="ones_row")
        nc.vector.memset(ones_row[:], 1.0)

        # resident weights (wq/wo DMAs deferred below so the first
        # ht tiles aren't starved on the shared DMA path)
        wk_sb = p1.tile([P, KS, HD], bf, tag="wk")
        nc.sync.dma_start(wk_sb[:], wkT_r[:])
        wv_sb = p1.tile([P, KS, HD], bf, tag="wv")
        wq_sb = p1.tile([P, KS, ADIM], bf, tag="wq")
        wo_sb = p1.tile([P, QH, HIDDEN], bf, tag="wo")

        # resident per-seq activations (single slot, reused across seqs)
        q_sb = p1.tile([P, QH, LMAX], bf, tag="q_sb")
        attn_A = p1.tile([P, QH, 1024], bf, tag="attn_A")
        attn_B = p1.tile([P, QH, LMAX], bf, tag="attn_B")
        kT_sb = p1.tile([P, LMAX], bf, tag="kT_sb")
        v_nat = p1.tile([P, LMAX // P, HD + 1], bf, tag="v_nat")

        # decode residents
        kT_dec = p1.tile([P, DECODE], bf, tag="kTdec")
        vdt = p1.tile([DECODE, HD], bf, tag="vdt")
        qdec_sb = p1.tile([P, P], bf, tag="qdec")      # cols (s,h)
        odec_sb = p1.tile([P, HD], bf, tag="odec")     # rows (s,h)
        attn_dec = p1.tile([P, QH, DECODE], bf, tag="attn_dec")

        qdec_r = qdec_sb.rearrange("p (s h) -> p s h", h=QH)

        # ---------------- phase 1: projections + rope ----------------
        def phase1_tile(t0, W, kT_dst, kcol0, v_dst, q_dst, qcol0,
                        dec=False, wq_dma=None, split_ht=1):
            H2 = KS // 2
            hta = p2.tile([P, H2, TW], bf, tag="hta")
            htb = p2.tile([P, H2, TW], bf, tag="htb")
            for half, ht_t in ((0, hta), (1, htb)):
                for i in range(split_ht):
                    sl = slice(i * H2 // split_ht, (i + 1) * H2 // split_ht)
                    sl2 = slice(half * H2 + i * H2 // split_ht,
                                half * H2 + (i + 1) * H2 // split_ht)
                    nc.sync.dma_start(ht_t[:, sl, :W], hT_r[:, sl2, t0:t0 + W])
            if split_ht > 1:
                nc.sync.dma_start(wv_sb[:], wvT_r[:])
            ct = p2.tile([P, TW], bf, tag="cos")
            st = p2.tile([P, TW], bf, tag="sin")
            nc.gpsimd.dma_start(ct[:, :W], qcos[:, t0:t0 + W])
            nc.gpsimd.dma_start(st[:, :W], qsin[:, t0:t0 + W])

            xq = p2.tile([P, NB, TW], bf, tag="xq")

            def group(w_ap, sink):
                ps = psBig.tile([P, 512], f32, tag="big")
                for ks in range(KS):
                    src = hta if ks < KS // 2 else htb
                    nc.tensor.matmul(
                        ps[:, :W], lhsT=w_ap[:, ks, :],
                        rhs=src[:, ks % (KS // 2), :W],
                        start=(ks == 0), stop=(ks == KS - 1),
                        skip_group_check=True)
                sink(ps)

            # k group -> xq row 0
            group(wk_sb, lambda ps: nc.scalar.activation(
                xq[:, 0, :W], ps[:, :W], AF.Copy))

            # v group -> transposes into v_dst
            def v_sink(ps):
                vt = p3.tile([P, TW], bf, tag="vt")
                nc.vector.tensor_copy(out=vt[:, :W], in_=ps[:, :W])
                if not dec:
                    for j in range(W // P):
                        pst = psSm.tile([P, P], bf, tag="sm")
                        nc.tensor.transpose(
                            pst[:], vt[:, j * P:(j + 1) * P], ident[:])
                        nc.vector.tensor_copy(
                            out=v_dst[:, kcol0 // P + j, :HD], in_=pst[:])
                else:
                    pst = psSm.tile([P, P], bf, tag="sm")
                    nc.tensor.transpose(pst[:DECODE, :], vt[:, :W], ident[:])
                    nc.vector.tensor_copy(out=v_dst[:], in_=pst[:DECODE, :])
            group(wv_sb, v_sink)

            if wq_dma is not None:
                wsrc, nchunks = wq_dma
                half = ADIM // nchunks
                for i in range(nchunks):
                    nc.sync.dma_start(
                        wq_sb[:, :, i * half:(i + 1) * half],
                        wsrc[:, :, i * half:(i + 1) * half])

            # q groups -> xq rows 1..4
            for m in range(QH):
                group(wq_sb[:, :, m * P:(m + 1) * P],
                      lambda ps, m=m: nc.scalar.activation(
                          xq[:, 1 + m, :W], ps[:, :W], AF.Copy))

            # rope on xq rows (k + 4 q)
            rotq = p2.tile([P, NB, TW], bf, tag="rotq")
            nc.gpsimd.dma_start(out=rotq[0:64, :, :W], in_=xq[64:128, :, :W])
            nc.gpsimd.dma_start(out=rotq[64:128, :, :W], in_=xq[0:64, :, :W])
            ct_b = ct[:, None, :W].to_broadcast((P, NB, W))
            st_b = st[:, None, :W].to_broadcast((P, NB, W))
            nc.vector.tensor_tensor(xq[:, :, :W], xq[:, :, :W], ct_b,
                                    ALU.mult)
            nc.vector.tensor_tensor(rotq[:, :, :W], rotq[:, :, :W], st_b,
                                    ALU.mult)
            nc.vector.tensor_tensor(
                kT_dst[:, kcol0:kcol0 + W], xq[:, 0, :W], rotq[:, 0, :W],
                ALU.add)
            nc.vector.tensor_tensor(
                q_dst[:, :, qcol0:qcol0 + W], xq[:, 1:NB, :W],
                rotq[:, 1:NB, :W], ALU.add)

        # ---------------- phase 4 unit generator ----------------
        NOMB = 2
        omb = {"buf": None, "fill": 0}
        ph4_in_window = [True]

        def phase4_units(out0, src0, L, src):
            """Yield once per m-block (4 matmuls + copy [+dma])."""
            for t0 in range(0, L, 512):
                W = min(512, L - t0)
                for mg in range(KS):
                    ps = psBig.tile([P, 512], f32, tag="big")
                    for ks in range(QH):
                        nc.tensor.matmul(
                            ps[:, :W], lhsT=wo_sb[:, ks, mg * P:(mg + 1) * P],
                            rhs=src[:, ks, src0 + t0:src0 + t0 + W],
                            start=(ks == 0), stop=(ks == QH - 1),
                            skip_group_check=True)
                    if omb["buf"] is None:
                        omb["n"] = omb.get("n", 0) + 1
                        omb["buf"] = p2.tile([P, NOMB, 512], bf, tag="omb",
                                             name=f"omb{omb['n']}")
                        omb["fill"] = 0
                    ob = omb["buf"]
                    slot = omb["fill"]
                    if ph4_in_window[0] or mg % 2 == 0:
                        nc.vector.tensor_copy(out=ob[:, slot, :W],
                                              in_=ps[:, :W])
                    else:
                        nc.scalar.activation(ob[:, slot, :W], ps[:, :W],
                                             AF.Copy)
                    omb["fill"] += 1
                    if omb["fill"] == NOMB:
                        nc.sync.dma_start(
                            outT_r[:, mg - NOMB + 1:mg + 1,
                                   out0 + t0:out0 + t0 + W],
                            ob[:, :, :W])
                        omb["buf"] = None
                    yield

        ph4_queue = []

        def pull_ph4(n):
            for _ in range(n):
                if not ph4_queue:
                    return
                try:
                    next(ph4_queue[0][1])
                except StopIteration:
                    ph4_queue.pop(0)

        def drain_older_than(idx):
            while ph4_queue and ph4_queue[0][0] < idx:
                try:
                    next(ph4_queue[0][1])
                except StopIteration:
                    ph4_queue.pop(0)

        # ---------------- decode units ----------------
        def prefetch_kd(s):
            kd = pDecK.tile([P, PAST], bf, tag="kd", name=f"kd{s}")
            nc.sync.dma_start(kd[:], kTc[s])
            return kd

        def prefetch_vd(s):
            vd = pDecV.tile([P, NKT_D, HD], bf, tag="vd", name=f"vd{s}")
            nc.sync.dma_start(vd[:], vcp[s].rearrange("p (kt d) -> p kt d",
                                                      d=HD))
            return vd

        dec_state = {"next": 0, "tiles": {}}

        def decode_emit_one():
            s = dec_state["next"]
            if s >= DECODE:
                return False
            dec_state["next"] += 1
            ks_, vs_ = dec_state.setdefault("kd", {}), dec_state.setdefault(
                "vd", {})
            if s == 0:
                ks_[0] = prefetch_kd(0)
                ks_[1] = prefetch_kd(1)
                vs_[0] = prefetch_vd(0)
            if s + 2 < DECODE:
                ks_[s + 2] = prefetch_kd(s + 2)
            if s + 1 < DECODE:
                vs_[s + 1] = prefetch_vd(s + 1)
            kd, vd = ks_.pop(s), vs_.pop(s)

            stp = psBig.tile([P, 72], f32, tag="big", name=f"stp{s}")
            for kt in range(NKT_D):
                nc.tensor.matmul(
                    stp[:, kt * QH:(kt + 1) * QH],
                    lhsT=kd[:, kt * P:(kt + 1) * P],
                    rhs=qdec_sb[:, s * QH:(s + 1) * QH],
                    start=True, stop=True, skip_group_check=True)
            nc.tensor.matmul(
                stp[0:1, 64:68], lhsT=kT_dec[:, s:s + 1],
                rhs=qdec_sb[:, s * QH:(s + 1) * QH],
                start=True, stop=True, skip_group_check=True)
            pt = p3.tile([P, 72], bf, tag="ptd")
            nc.scalar.activation(pt[:, :64], stp[:, :64], AF.Exp, scale=SCALE)
            nc.scalar.activation(pt[0:1, 64:68], stp[0:1, 64:68], AF.Exp,
                                 scale=SCALE)

            vrow = p3.tile([1, HD], bf, tag="vrow")
            nc.gpsimd.dma_start(out=vrow[:], in_=vdt[s:s + 1, :])
            # PV transposed: out [128 d, 4 h] so each matmul is 4 rows
            ov = psBig.tile([P, QH], f32, tag="big", name=f"ov{s}")
            ovs = psSm.tile([1, QH], f32, tag="sm", name=f"ovs{s}")
            for kt in range(NKT_D):
                nc.tensor.matmul(
                    ov[:], lhsT=vd[:, kt, :],
                    rhs=pt[:, kt * QH:(kt + 1) * QH],
                    start=(kt == 0), stop=False, skip_group_check=True)
                nc.tensor.matmul(
                    ovs[:], lhsT=ones_sb[:],
                    rhs=pt[:, kt * QH:(kt + 1) * QH],
                    start=(kt == 0), stop=False, skip_group_check=True)
            nc.tensor.matmul(ov[:], lhsT=vrow[:],
                             rhs=pt[0:1, 64:68], start=False, stop=True,
                             skip_group_check=True)
            nc.tensor.matmul(ovs[:], lhsT=ones_sb[0:1, :],
                             rhs=pt[0:1, 64:68], start=False, stop=True,
                             skip_group_check=True)
            # reciprocal row, then PE-broadcast it down all 128 partitions
            rrow = p3.tile([1, QH], bf, tag="rrow")
            with nc.allow_low_precision(reason="bf16 softmax denom"):
                nc.vector.reciprocal(rrow[:], ovs[:])
            rbc = psSm.tile([P, QH], f32, tag="sm", name=f"rbc{s}")
            nc.tensor.matmul(rbc[:], lhsT=ones_row[:], rhs=rrow[:],
                             start=True, stop=True, skip_group_check=True)
            rbs = p3.tile([P, QH], bf, tag="rbs")
            nc.vector.tensor_copy(out=rbs[:], in_=rbc[:])
            nc.vector.tensor_tensor(attn_dec[:, :, s], ov[:], rbs[:],
                                    ALU.mult)
            if s == DECODE - 1:
                ph4_queue.append(
                    (98, phase4_units(DOFF, 0, DECODE, attn_dec)))
            return True

        def enqueue_ph4(idx, si, t0, L):
            s0, _ = SEQ_BOUNDS[si]
            slot = attn_A if idx % 2 == 0 else attn_B
            ph4_queue.append((idx, phase4_units(s0 + t0, t0, L, slot)))

        # ---------------- phase 2 window ----------------
        def window(idx, si):
            s0, L = SEQ_BOUNDS[si]
            attn_sb = attn_A if idx % 2 == 0 else attn_B
            B = L // P
            debt = [0.0]

            def fill(ns):
                debt[0] += ns
                while debt[0] >= 853.0 and ph4_queue:
                    try:
                        next(ph4_queue[0][1])
                        debt[0] -= 853.0
                    except StopIteration:
                        ph4_queue.pop(0)

            prev = {"pos": None, "Q0": 0, "obufs": None}

            def emit_norm():
                # previous block's softmax normalization (DVE only)
                obufs = []
                for h in range(QH):
                    rr = p3.tile([P, 1], f32, tag="rr")
                    nc.vector.reciprocal(rr[:], prev["pos"][h][:, HD:HD + 1])
                    obuf = p4.tile([P, P], bf, tag="obuf",
                                   name=f"ob{si}_{prev['Q0']}_{h}")
                    nc.vector.tensor_scalar_mul(
                        obuf[:], prev["pos"][h][:, :HD], rr[:])
                    obufs.append(obuf)
                prev["obufs"] = obufs
                prev["pos"] = None

            def emit_transposes():
                Q0p = prev["Q0"]
                for h in range(QH):
                    pst = psSm.tile([P, P], bf, tag="sm")
                    nc.tensor.transpose(pst[:], prev["obufs"][h][:], ident[:])
                    if h % 2 == 0:
                        nc.vector.tensor_copy(
                            out=attn_sb[:, h, Q0p:Q0p + P], in_=pst[:])
                    else:
                        nc.scalar.activation(
                            attn_sb[:, h, Q0p:Q0p + P], pst[:], AF.Copy)
                prev["obufs"] = None
                if (Q0p // P + 1) % 4 == 0:
                    enqueue_ph4(idx, si, Q0p - 3 * P, 512)

            for qb in range(B):
                Q0 = qb * P
                pos = [psPo.tile([P, HD + 1], f32, tag=f"po{h}",
                                 name=f"po{h}_{si}_{qb}") for h in range(QH)]
                q4 = q_sb[:, :, Q0:Q0 + P]
                pending = []   # (pch, c) with exp done, PV not yet emitted

                def emit_pv(pch, c, qb=qb, pos=pos):
                    for h in range(QH):
                        nc.tensor.matmul(
                            pos[h][:], lhsT=pch[:, h, :],
                            rhs=v_nat[:, c, :], start=(c == 0),
                            stop=(c == qb), skip_group_check=True)

                for c in range(qb + 1):
                    sps = psBig.tile([P, 512], f32, tag="big")
                    nc.tensor.matmul(sps[:], lhsT=kT_sb[:, c * P:(c + 1) * P],
                                     rhs=q4, start=True, stop=True,
                                     skip_group_check=True)
                    pch = p4.tile([P, QH, P], bf, tag="pch")
                    nc.scalar.activation(pch[:], sps[:], AF.Exp, scale=SCALE)
                    if c == qb:
                        nc.gpsimd.affine_select(
                            out=pch[:], in_=pch[:], compare_op=ALU.is_ge,
                            fill=0.0, base=0, channel_multiplier=-1,
                            pattern=[[0, QH], [1, P]])
                    if c == 0 and prev["pos"] is not None:
                        emit_norm()
                    if len(pending) >= 2:
                        emit_pv(*pending.pop(0))
                    pending.append((pch, c))
                    if c == 1 and prev["obufs"] is not None:
                        emit_transposes()
                    fill(450.0)
                while pending:
                    emit_pv(*pending.pop(0))
                if prev["obufs"] is not None:   # block had < 2 chunks
                    emit_transposes()
                prev["pos"] = pos
                prev["Q0"] = Q0
            # final block: norm + transposes at window end
            emit_norm()
            emit_transposes()

        # ---------------- program ----------------
        # ones column of v_nat (rewritten per seq by transposes on :HD only)
        nc.vector.memset(v_nat[:, :, HD:HD + 1], 1.0)

        # seq0 tile 0 first (k/v groups run on wk/wv alone), weight DMAs
        # threaded between so the shared DMA path never starves PE
        qd_tmp = p1.tile([P, QH, DECODE], bf, tag="qd_tmp")
        phase1_tile(0, TW, kT_sb, 0, v_nat, q_sb, 0,
                    wq_dma=(wqT_r, 2), split_ht=4)
        phase1_tile(DOFF, DECODE, kT_dec, 0, vdt, qd_tmp, 0, dec=True)
        for h in range(QH):
            nc.gpsimd.dma_start(out=qdec_r[:, :, h], in_=qd_tmp[:, h, :])

        first_tile_done = True
        for idx, si in enumerate([0, 1, 3, 2]):
            s0, L = SEQ_BOUNDS[si]
            for lt in range(L // TW):
                if si == 0 and lt == 0:
                    continue
                phase1_tile(s0 + lt * TW, TW, kT_sb, lt * TW, v_nat,
                            q_sb, lt * TW)
                decode_emit_one()
                decode_emit_one()
                if lt == 1 and idx == 0:
                    nc.sync.dma_start(wo_sb[:], woT_r[:])
            # phase4 of the seq that used this window's attn slot last
            # must be fully drained before this window's first attn write
            ph4_in_window[0] = False
            drain_older_than(idx - 1)
            ph4_in_window[0] = True
            window(idx, si)

        ph4_in_window[0] = False
        while ph4_queue:
            pull_ph4(64)
        ph4_in_window[0] = True

        # remaining decode units (assembly handled inside the last unit)
        while decode_emit_one():
            pass
        while ph4_queue:
            pull_ph4(64)

    nc.compile()
    return nc


_NC = None


def _get_program():
    global _NC
    if _NC is None:
        _NC = build_program()
    return _NC


def _rope_tables():
    inv_freq = 1.0 / (10000.0 ** (np.arange(0, HD, 2, dtype=np.float32) / HD))
    pos_q = np.concatenate(
        [np.arange(L, dtype=np.float32) for L in PREFILLS]
        + [np.full(DECODE, float(PAST), np.float32)])                 # [T]
    ang_q = np.outer(inv_freq, pos_q)                                 # [64, T]
    qcos = np.concatenate([np.cos(ang_q), np.cos(ang_q)], axis=0)
    qsin = np.concatenate([-np.sin(ang_q), np.sin(ang_q)], axis=0)
    return qcos.astype(BF16), qsin.astype(BF16)


def _rope_cache(k):
    # k: [DECODE, PAST, HD] float32; positions 0..PAST-1
    inv_freq = 1.0 / (10000.0 ** (np.arange(0, HD, 2, dtype=np.float32) / HD))
    ang = np.outer(np.arange(PAST, dtype=np.float32), inv_freq)  # [PAST, 64]
    cos = np.concatenate([np.cos(ang), np.cos(ang)], axis=1)     # [PAST, 128]
    sin = np.concatenate([np.sin(ang), np.sin(ang)], axis=1)
    rot = np.concatenate([-k[..., HD // 2:], k[..., :HD // 2]], axis=-1)
    return k * cos[None] + rot * sin[None]


def make_in_maps(hidden_states, wq, wk, wv, wo, kv_cache_k, kv_cache_v):
    hidden_states = np.asarray(hidden_states)
    wq, wk, wv, wo = (np.asarray(a) for a in (wq, wk, wv, wo))
    kv_cache_k, kv_cache_v = np.asarray(kv_cache_k), np.asarray(kv_cache_v)

    hT = np.ascontiguousarray(hidden_states.astype(BF16).T)      # [4096, T]
    qcos, qsin = _rope_tables()
    in_maps = []
    for c in range(NCORES):
        wqT = np.ascontiguousarray(wq[c * ADIM:(c + 1) * ADIM, :]
                                   .astype(BF16).T)
        wkT = np.ascontiguousarray(wk[c * HD:(c + 1) * HD, :].astype(BF16).T)
        wvT = np.ascontiguousarray(wv[c * HD:(c + 1) * HD, :].astype(BF16).T)
        woT = np.ascontiguousarray(wo[:, c * ADIM:(c + 1) * ADIM]
                                   .astype(BF16).T)
        kc = _rope_cache(kv_cache_k[:, :, c, :].astype(np.float32))
        kTc = np.ascontiguousarray(
            kc.astype(BF16).transpose(0, 2, 1))                  # [32,128,2048]
        vcc = kv_cache_v[:, :, c, :].astype(BF16)                # [32,2048,128]
        vcp = np.ascontiguousarray(
            vcc.reshape(DECODE, NKT_D, P, HD).transpose(0, 2, 1, 3)
            .reshape(DECODE, P, PAST))
        in_maps.append({
            "hT": hT, "wqT": wqT, "wkT": wkT, "wvT": wvT, "woT": woT,
            "kTc": kTc, "vcp": vcp, "qcos": qcos, "qsin": qsin,
        })
    return in_maps


def combine_outputs(results):
    acc = np.zeros((HIDDEN, T), np.float32)
    for c in range(NCORES):
        acc += results[c]["outT"].astype(np.float32)
    return np.ascontiguousarray(acc.T)


def kernel(hidden_states, wq, wk, wv, wo, kv_cache_k, kv_cache_v):
    from concourse.bass_utils import run_bass_kernel_spmd

    nc = _get_program()
    in_maps = make_in_maps(hidden_states, wq, wk, wv, wo,
                           kv_cache_k, kv_cache_v)
    res = run_bass_kernel_spmd(nc, in_maps, core_ids=list(range(NCORES)))
    return combine_outputs(res.results)
